# revision 6
# baseline (speedup 1.0000x reference)
"""Trainium2 Bass kernel for a cross-attention transformer block.

Contract: kernel(**inputs) takes the FULL inputs (B=8 batch), shards
batch-wise across 8 NeuronCores (one batch element per core, SPMD, no
collectives), runs a Bass/Tile kernel, and returns the FULL output.

Per-core pipeline (everything stored feature-major, "X^T" [feat, tok],
so every linear is a single PE matmul pass with no transposes):
  Qp^T = (Wq/8)^T q^T   Kp^T = Wk^T k^T    (transposed-layout projections)
  Vp   = v^T-tiles as lhsT against Wv      (natural-layout projection)
  S^T  = Kp_h^T . Qp_h  per head (K=64, two heads row-packed in the PE)
  p    = exp(S^T + colNEG[k])              (ACT, per-partition bias; no max
                                            subtraction needed: |s| <~ 5)
  out^T= [Vp | 1] @ p                      (M=65: row 64 = softmax denom)
       + rank-1 corrections for masked query rows (exact, via K=1 matmuls)
  mha  = Wo-projection done twice (transposed for the FFN input, natural
         for the residual), FFN with fused relu+bias, residual + LayerNorm
         in natural layout, DMA out.

Numerics: fp32r matmuls (FP22 multiply / fp32 accumulate) ~1.5e-4 rel.
"""

import os
import sys

for _p in ("/opt/trn_rl_repo",):
    if _p not in sys.path:
        sys.path.insert(0, _p)

import numpy as np

import concourse.bacc as bacc
import concourse.tile as tile
from concourse import mybir

F32 = mybir.dt.float32
F32R = mybir.dt.float32r
AF = mybir.ActivationFunctionType
OP = mybir.AluOpType

D = 768
H = 12
HD = 64
DT = 6          # feature tiles of 128
L = 1024
NEG = -1000000.0
EPS = 1e-5

_CHUNKS = {
    256: [256], 384: [384], 512: [512], 640: [384, 256], 768: [512, 256],
    896: [512, 384], 1024: [512, 512],
}


def _chunks(width):
    out, off = [], 0
    for w in _CHUNKS[width]:
        out.append((off, w))
        off += w
    return out


def _pad128(n):
    return int(min(L, max(256, ((int(n) + 127) // 128) * 128)))


def build_program(k_pad, q_pad, n_cores):
    kt_n = k_pad // 128
    qch = _chunks(q_pad)
    kch = _chunks(k_pad)
    tail = L - q_pad  # rank-1b region width (may be 0)

    nc = bacc.Bacc("TRN2", target_bir_lowering=False, debug=False,
                   num_devices=n_cores)

    def din(name, shape, dt=F32):
        return nc.dram_tensor(name, shape, dt, kind="ExternalInput").ap()

    qT = din("qT", [D, q_pad])
    kT = din("kT", [D, k_pad])
    vT = din("vT", [D, k_pad])
    wq = din("wq", [D, D])
    wk = din("wk", [D, D])
    wv = din("wv", [D, D])
    wo = din("wo", [D, D])
    d1w = din("d1w", [D, D])
    d2w = din("d2w", [D, D])
    vauxm = din("vauxm", [128, DT])
    vauxa = din("vauxa", [128, DT])
    colneg = din("colneg", [128, kt_n])
    wvec = din("wvec", [1, q_pad])
    sigu = din("sigu", [1, H])
    d1b = din("d1b", [128, DT])
    d2b = din("d2b", [1, D])
    lng = din("lng", [1, D])
    lnb = din("lnb", [1, D])
    out = nc.dram_tensor("out", [L, D], F32, kind="ExternalOutput").ap()

    from contextlib import ExitStack
    with tile.TileContext(nc) as tc, ExitStack() as ctx:
        # ---------------- long-lived small tiles ----------------
        plong = ctx.enter_context(tc.tile_pool(name="plong", bufs=1))
        colneg_s = plong.tile([128, kt_n], F32, name="colneg_s")
        nc.sync.dma_start(out=colneg_s[:], in_=colneg[:, :])
        wvec_s = plong.tile([1, q_pad], F32R, name="wvec_s")
        nc.sync.dma_start(out=wvec_s[:], in_=wvec[:, :].bitcast(F32R))
        ones_s = plong.tile([1, 512], F32R, name="ones_s")
        ONE_BITS = 0x3F800000  # walrus rejects float32r memset; write bits
        nc.vector.memset(ones_s[:].bitcast(mybir.dt.uint32), ONE_BITS)
        vm65row = plong.tile([1, 65 * H], F32R, name="vm65row")
        va65row = plong.tile([1, 65 * H], F32R, name="va65row")
        vauxm_s = plong.tile([128, DT], F32R, name="vauxm_s")
        nc.sync.dma_start(out=vauxm_s[:], in_=vauxm[:, :].bitcast(F32R))
        vauxa_s = plong.tile([128, DT], F32R, name="vauxa_s")
        nc.sync.dma_start(out=vauxa_s[:], in_=vauxa[:, :].bitcast(F32R))
        gb = plong.tile([128, D], F32, name="gb")
        nc.sync.dma_start(out=gb[:], in_=lng.to_broadcast([128, D]))
        bb = plong.tile([128, D], F32, name="bb")
        nc.sync.dma_start(out=bb[:], in_=lnb.to_broadcast([128, D]))
        d2bb = plong.tile([128, D], F32, name="d2bb")
        nc.sync.dma_start(out=d2bb[:], in_=d2b.to_broadcast([128, D]))
        epst = plong.tile([128, 1], F32, name="epst")
        nc.vector.memset(epst[:], EPS)
        d1b_s = plong.tile([128, DT], F32, name="d1b_s")
        nc.sync.dma_start(out=d1b_s[:], in_=d1b[:, :])

        # attnorm^T lives from attention through the Wo projections
        sBC = ExitStack()
        pbc = sBC.enter_context(tc.tile_pool(name="pbc", bufs=1))
        attnorm = [pbc.tile([128, L], F32R, name=f"attnorm{j}")
                   for j in range(DT)]

        # ---------------- phase A+B scope ----------------
        sAB = ExitStack()
        pproj = sAB.enter_context(tc.tile_pool(name="pproj", bufs=1))
        Qp = [pproj.tile([128, q_pad], F32R, name=f"Qp{j}") for j in range(DT)]
        Kp = [pproj.tile([128, k_pad], F32R, name=f"Kp{j}") for j in range(DT)]
        Vm65 = [pproj.tile([128, 65 * H], F32R, name=f"Vm65_{k}")
                for k in range(kt_n)]
        # ---------------- phase A: projections ----------------
        sA = ExitStack()
        pin = sA.enter_context(tc.tile_pool(name="pin", bufs=1))
        pw = sA.enter_context(tc.tile_pool(name="pw", bufs=3))
        psA = sA.enter_context(tc.tile_pool(name="psA", bufs=1, space="PSUM"))

        qTs = [pin.tile([128, q_pad], F32R, name=f"qTs{t}") for t in range(DT)]
        kTs = [pin.tile([128, k_pad], F32R, name=f"kTs{t}") for t in range(DT)]
        vTs = [pin.tile([128, k_pad], F32R, name=f"vTs{t}") for t in range(DT)]
        for t in range(DT):
            r = slice(128 * t, 128 * t + 128)
            nc.sync.dma_start(out=qTs[t][:], in_=qT[r, :].bitcast(F32R))
            nc.sync.dma_start(out=kTs[t][:], in_=kT[r, :].bitcast(F32R))
            nc.sync.dma_start(out=vTs[t][:], in_=vT[r, :].bitcast(F32R))
        wvs = [pin.tile([128, D], F32R, name=f"wvs{t}") for t in range(DT)]
        for t in range(DT):
            nc.sync.dma_start(out=wvs[t][:],
                              in_=wv[128 * t:128 * t + 128, :].bitcast(F32R))

        # Q/K projections in transposed layout, two dout tiles at a time
        for (wdram, xs, outts, chs) in ((wq, qTs, Qp, qch), (wk, kTs, Kp, kch)):
            for jh in range(3):
                pss = {}
                for jj in range(2):
                    for (c0, cw) in chs:
                        pss[jj, c0] = psA.tile(
                            [128, cw], F32, tag="A", bufs=4,
                            name=f"psA_{id(wdram)%97}_{jh}_{jj}_{c0}",
                            padded_shape=[128, 768])
                for t in range(DT):
                    wt = pw.tile([128, 256], F32R, tag="wst",
                                 name=f"w_{id(wdram)%97}_{jh}_{t}")
                    nc.sync.dma_start(
                        out=wt[:],
                        in_=wdram[128 * t:128 * t + 128,
                                  256 * jh:256 * jh + 256].bitcast(F32R))
                    for jj in range(2):
                        for (c0, cw) in chs:
                            nc.tensor.matmul(
                                pss[jj, c0][:, :],
                                wt[:, 128 * jj:128 * jj + 128],
                                xs[t][:, c0:c0 + cw],
                                start=(t == 0), stop=(t == DT - 1))
                for jj in range(2):
                    j = 2 * jh + jj
                    for (c0, cw) in chs:
                        nc.scalar.copy(out=outts[j][:, c0:c0 + cw],
                                       in_=pss[jj, c0][:, :])

        # V projection in natural layout -> Vm65 (65-stride gaps per head)
        for kt in range(kt_n):
            psv = psA.tile([128, D], F32, tag="A", bufs=4, name=f"psV{kt}",
                           padded_shape=[128, 768])
            for t in range(DT):
                for (n0, nw) in ((0, 512), (512, 256)):
                    nc.tensor.matmul(
                        psv[:, n0:n0 + nw],
                        vTs[t][:, 128 * kt:128 * kt + 128],
                        wvs[t][:, n0:n0 + nw],
                        start=(t == 0), stop=(t == DT - 1))
            src = psv[:, :].rearrange("p (h e) -> p h e", e=64)
            dst = Vm65[kt][:].rearrange("p (h e) -> p h e", e=65)[:, :, 0:64]
            nc.vector.tensor_copy(out=dst, in_=src)
            nc.vector.memset(
                Vm65[kt][:].bitcast(mybir.dt.uint32)
                .rearrange("p (h e) -> p h e", e=65)[:, :, 64:65], ONE_BITS)

        # aux sums: (sum of masked v rows) @ Wv and (sum of all v rows) @ Wv
        for (aux_s, rowt, scale) in ((vauxm_s, vm65row, 1.0),
                                     (vauxa_s, va65row, 1.0 / L)):
            psx = psA.tile([1, D], F32, tag="A", bufs=4,
                           name=f"psaux{scale!r}", padded_shape=[128, 768])
            for t in range(DT):
                for (n0, nw) in ((0, 512), (512, 256)):
                    nc.tensor.matmul(
                        psx[:, n0:n0 + nw], aux_s[:, t:t + 1],
                        wvs[t][:, n0:n0 + nw],
                        start=(t == 0), stop=(t == DT - 1))
            nc.scalar.mul(
                out=rowt[:].rearrange("p (h e) -> p h e", e=65)[:, :, 0:64],
                in_=psx[0:1, :].rearrange("p (h e) -> p h e", e=64),
                mul=scale)
        nc.sync.dma_start(
            out=vm65row[:].rearrange("p (h e) -> p h e", e=65)[:, :, 64:65],
            in_=sigu[:, :].bitcast(F32R).rearrange("p (h e) -> p h e", e=1))
        nc.vector.memset(
            va65row[:].bitcast(mybir.dt.uint32)
            .rearrange("p (h e) -> p h e", e=65)[:, :, 64:65], ONE_BITS)

        sA.close()

        # ---------------- phase B: attention ----------------
        ppexp = sAB.enter_context(tc.tile_pool(name="ppexp", bufs=4))
        pden = sAB.enter_context(tc.tile_pool(name="pden", bufs=1))
        psB = sAB.enter_context(tc.tile_pool(name="psB", bufs=1, space="PSUM"))
        for h in range(H):
            jt, po = h // 2, 64 * (h % 2)
            hs = slice(65 * h, 65 * h + 65)
            for (q0, qw) in qch:
                ao = psB.tile([65, qw], F32, tag="ao", bufs=3,
                              name=f"ao{h}_{q0}", padded_shape=[65, 512])
                for kt in range(kt_n):
                    sc = psB.tile([128, qw], F32, tag="sc", bufs=3,
                                  name=f"sc{h}_{q0}_{kt}",
                                  padded_shape=[128, 512])
                    nc.tensor.matmul(
                        sc[:, :],
                        Kp[jt][po:po + 64, 128 * kt:128 * kt + 128],
                        Qp[jt][po:po + 64, q0:q0 + qw],
                        start=True, stop=True)
                    p = ppexp.tile([128, qw], F32R, tag="p", bufs=4,
                                   name=f"p{h}_{q0}_{kt}",
                                   padded_shape=[128, 512])
                    nc.scalar.activation(out=p[:], in_=sc[:, :], func=AF.Exp,
                                         bias=colneg_s[:, kt:kt + 1],
                                         scale=1.0)
                    nc.tensor.matmul(ao[:, :], Vm65[kt][:, hs], p[:],
                                     start=(kt == 0), stop=False)
                nc.tensor.matmul(ao[:, :], vm65row[0:1, hs],
                                 wvec_s[0:1, q0:q0 + qw],
                                 start=False, stop=True)
                dn = pden.tile([1, qw], F32, tag="dn", bufs=4,
                               name=f"dn{h}_{q0}", padded_shape=[1, 512])
                nc.vector.tensor_copy(out=dn[:], in_=ao[64:65, :])
                rc = pden.tile([1, qw], F32R, tag="rc", bufs=4,
                               name=f"rc{h}_{q0}", padded_shape=[1, 512])
                with nc.allow_low_precision(
                        reason="f32r annotation only; fp22 recip is ample"):
                    nc.vector.reciprocal(out=rc[:], in_=dn[:])
                rbp = psB.tile([64, qw], F32, tag="rb", bufs=2,
                               name=f"rbp{h}_{q0}", padded_shape=[64, 512])
                nc.tensor.matmul(rbp[:, :], ones_s[0:1, 0:64], rc[:],
                                 start=True, stop=True)
                rbs = pden.tile([64, qw], F32, tag="rbs", bufs=3,
                                name=f"rbs{h}_{q0}", padded_shape=[64, 512])
                nc.scalar.copy(out=rbs[:], in_=rbp[:, :])
                nc.vector.tensor_tensor(
                    out=attnorm[jt][po:po + 64, q0:q0 + qw],
                    in0=ao[0:64, :], in1=rbs[:], op=OP.mult)
            if tail:
                ao2 = psB.tile([65, tail], F32, tag="ao", bufs=3,
                               name=f"ao2_{h}", padded_shape=[65, 512])
                nc.tensor.matmul(ao2[:, :], va65row[0:1, hs],
                                 ones_s[0:1, 0:tail], start=True, stop=True)
                nc.scalar.copy(out=attnorm[jt][po:po + 64, q_pad:L],
                               in_=ao2[0:64, :])

        sAB.close()

        # ---------------- phase C: Wo both layouts ----------------
        sCD = ExitStack()
        pcd = sCD.enter_context(tc.tile_pool(name="pcd", bufs=1, side="right"))
        mhaT = [pcd.tile([128, L], F32R, name=f"mhaT{j}") for j in range(DT)]
        mhaN = [pcd.tile([128, D], F32, name=f"mhaN{q}") for q in range(8)]

        sC = ExitStack()
        pc_w = sC.enter_context(tc.tile_pool(name="pc_w", bufs=1))
        psC = sC.enter_context(tc.tile_pool(name="psC", bufs=1, space="PSUM"))
        wos = [pc_w.tile([128, D], F32R, name=f"wos{t}") for t in range(DT)]
        for t in range(DT):
            nc.sync.dma_start(out=wos[t][:],
                              in_=wo[128 * t:128 * t + 128, :].bitcast(F32R))
        for j in range(DT):
            for q0 in (0, 512):
                ps = psC.tile([128, 512], F32, tag="C", bufs=3,
                              name=f"psT{j}_{q0}", padded_shape=[128, 768])
                for t in range(DT):
                    nc.tensor.matmul(ps[:, :],
                                     wos[t][:, 128 * j:128 * j + 128],
                                     attnorm[t][:, q0:q0 + 512],
                                     start=(t == 0), stop=(t == DT - 1))
                nc.scalar.copy(out=mhaT[j][:, q0:q0 + 512], in_=ps[:, :])
        for qi in range(8):
            ps = psC.tile([128, D], F32, tag="C", bufs=3, name=f"psN{qi}",
                          padded_shape=[128, 768])
            for (n0, nw) in ((0, 512), (512, 256)):
                for t in range(DT):
                    nc.tensor.matmul(ps[:, n0:n0 + nw],
                                     attnorm[t][:, 128 * qi:128 * qi + 128],
                                     wos[t][:, n0:n0 + nw],
                                     start=(t == 0), stop=(t == DT - 1))
            # residual input: mha (natural) + d2 bias, ready for x = ffn + .
            nc.vector.tensor_tensor(out=mhaN[qi][:], in0=ps[:, :],
                                    in1=d2bb[:], op=OP.add)
        sC.close()
        sBC.close()

        # ---------------- phase D: FFN + residual + LayerNorm ----------------
        sD = ExitStack()
        pd_w = sD.enter_context(tc.tile_pool(name="pd_w", bufs=1, side="right"))
        pdst = sD.enter_context(tc.tile_pool(name="pdst", bufs=3, side="right"))
        pdx = sD.enter_context(tc.tile_pool(name="pdx", bufs=1, side="right"))
        psmall = sD.enter_context(tc.tile_pool(name="psmall", bufs=8, side="right"))
        psD = sD.enter_context(tc.tile_pool(name="psD", bufs=1, space="PSUM"))

        reluT = [pdx.tile([128, L], F32R, name=f"reluT{j}") for j in range(DT)]
        d2s = [pd_w.tile([128, D], F32R, name=f"d2s{t}") for t in range(DT)]
        for t in range(DT):
            nc.sync.dma_start(out=d2s[t][:],
                              in_=d2w[128 * t:128 * t + 128, :].bitcast(F32R))

        for j in range(DT):
            for q0 in (0, 512):
                ps = psD.tile([128, 512], F32, tag="D", bufs=3,
                              name=f"psd1_{j}_{q0}", padded_shape=[128, 768])
                for t in range(DT):
                    wt = pdst.tile([128, 128], F32R, tag="d1st",
                                   name=f"d1_{j}_{q0}_{t}")
                    nc.sync.dma_start(
                        out=wt[:],
                        in_=d1w[128 * t:128 * t + 128,
                                128 * j:128 * j + 128].bitcast(F32R))
                    nc.tensor.matmul(ps[:, :], wt[:],
                                     mhaT[t][:, q0:q0 + 512],
                                     start=(t == 0), stop=(t == DT - 1))
                nc.scalar.activation(out=reluT[j][:, q0:q0 + 512],
                                     in_=ps[:, :], func=AF.Relu,
                                     bias=d1b_s[:, j:j + 1], scale=1.0)

        inv_d = 1.0 / D
        for qi in range(8):
            ps = psD.tile([128, D], F32, tag="D", bufs=3, name=f"psff{qi}",
                          padded_shape=[128, 768])
            for (n0, nw) in ((0, 512), (512, 256)):
                for t in range(DT):
                    nc.tensor.matmul(ps[:, n0:n0 + nw],
                                     reluT[t][:, 128 * qi:128 * qi + 128],
                                     d2s[t][:, n0:n0 + nw],
                                     start=(t == 0), stop=(t == DT - 1))
            x = pdx.tile([128, D], F32, tag="x", bufs=2, name=f"x{qi}")
            nc.vector.tensor_tensor(out=x[:], in0=ps[:, :], in1=mhaN[qi][:],
                                    op=OP.add)
            xsum = psmall.tile([128, 1], F32, tag="s1", name=f"xsum{qi}")
            nc.vector.tensor_reduce(out=xsum[:], in_=x[:],
                                    axis=mybir.AxisListType.X, op=OP.add)
            scr = pdx.tile([128, D], F32, tag="scr", bufs=2, name=f"scr{qi}")
            xsq = psmall.tile([128, 1], F32, tag="s2", name=f"xsq{qi}")
            nc.scalar.activation(out=scr[:], in_=x[:], func=AF.Square,
                                 accum_out=xsq[:])
            mu = psmall.tile([128, 1], F32, tag="s3", name=f"mu{qi}")
            nc.vector.tensor_scalar_mul(out=mu[:], in0=xsum[:], scalar1=inv_d)
            var = psmall.tile([128, 1], F32, tag="s4", name=f"var{qi}")
            # var = xsq/D - mu^2  ==  (xsq*1/D) - mu*mu
            mu2 = psmall.tile([128, 1], F32, tag="s5", name=f"mu2{qi}")
            nc.vector.tensor_tensor(out=mu2[:], in0=mu[:], in1=mu[:],
                                    op=OP.mult)
            nc.vector.scalar_tensor_tensor(out=var[:], in0=xsq[:],
                                           scalar=inv_d, in1=mu2[:],
                                           op0=OP.mult, op1=OP.subtract)
            std = psmall.tile([128, 1], F32, tag="s6", name=f"std{qi}")
            nc.scalar.activation(out=std[:], in_=var[:], func=AF.Sqrt,
                                 bias=epst[:], scale=1.0)
            rstd = psmall.tile([128, 1], F32, tag="s7", name=f"rstd{qi}")
            nc.vector.reciprocal(out=rstd[:], in_=std[:])
            nmb = psmall.tile([128, 1], F32, tag="s8", name=f"nmb{qi}")
            nc.vector.scalar_tensor_tensor(out=nmb[:], in0=mu[:], scalar=-1.0,
                                           in1=rstd[:], op0=OP.mult,
                                           op1=OP.mult)
            # xn = x*rstd + (-mu*rstd); then *g + b
            nc.vector.tensor_scalar(out=scr[:], in0=x[:], scalar1=rstd[:],
                                    scalar2=nmb[:], op0=OP.mult, op1=OP.add)
            nc.vector.tensor_tensor(out=x[:], in0=scr[:], in1=gb[:],
                                    op=OP.mult)
            xo = pdx.tile([128, D], F32, tag="xo", bufs=2, name=f"xo{qi}")
            nc.vector.tensor_tensor(out=xo[:], in0=x[:], in1=bb[:], op=OP.add)
            nc.sync.dma_start(out=out[128 * qi:128 * qi + 128, :], in_=xo[:])
        sD.close()
        sCD.close()

    nc.compile()
    return nc


_PROGRAM_CACHE = {}


def _get_program(k_pad, q_pad, n_cores):
    key = (k_pad, q_pad, n_cores)
    if key not in _PROGRAM_CACHE:
        _PROGRAM_CACHE[key] = build_program(k_pad, q_pad, n_cores)
    return _PROGRAM_CACHE[key]


def make_in_map(b, k_pad, q_pad, queries, keys, values, mask_1, mask_2,
                Wq, Wk, Wv, Wo, d1_w, d1_b, d2_w, d2_b, ln_g, ln_b):
    kt_n = k_pad // 128
    f32 = np.float32
    vl1 = int(np.count_nonzero(mask_1[b]))
    vl2 = int(np.count_nonzero(mask_2[b]))
    row01 = (np.arange(L) < vl2).astype(f32)
    qmask = np.asarray(queries[b], f32) * row01[:, None]
    col01 = (np.arange(L) < vl1)
    cn = np.where(col01, 0.0, NEG).astype(f32)[:k_pad]
    vb = np.asarray(values[b], f32)
    vinmasked = vb[vl1:, :].sum(0, dtype=np.float64).astype(f32)
    vinall = vb.sum(0, dtype=np.float64).astype(f32)
    return {
        "qT": np.ascontiguousarray(qmask.T[:, :q_pad]),
        "kT": np.ascontiguousarray(np.asarray(keys[b], f32).T[:, :k_pad]),
        "vT": np.ascontiguousarray(vb.T[:, :k_pad]),
        "wq": np.ascontiguousarray(np.asarray(Wq, f32) * 0.125),
        "wk": np.ascontiguousarray(np.asarray(Wk, f32)),
        "wv": np.ascontiguousarray(np.asarray(Wv, f32)),
        "wo": np.ascontiguousarray(np.asarray(Wo, f32)),
        "d1w": np.ascontiguousarray(np.asarray(d1_w, f32)),
        "d2w": np.ascontiguousarray(np.asarray(d2_w, f32)),
        "vauxm": np.ascontiguousarray(vinmasked.reshape(DT, 128).T),
        "vauxa": np.ascontiguousarray(vinall.reshape(DT, 128).T),
        "colneg": np.ascontiguousarray(cn.reshape(kt_n, 128).T),
        "wvec": np.ascontiguousarray((1.0 - row01)[None, :q_pad]),
        "sigu": np.full((1, H), float(L - vl1), f32),
        "d1b": np.ascontiguousarray(np.asarray(d1_b, f32).reshape(DT, 128).T),
        "d2b": np.ascontiguousarray(np.asarray(d2_b, f32)[None, :]),
        "lng": np.ascontiguousarray(np.asarray(ln_g, f32)[None, :]),
        "lnb": np.ascontiguousarray(np.asarray(ln_b, f32)[None, :]),
    }


def kernel(queries, keys, values, mask_1, mask_2,
           Wq, Wk, Wv, Wo, d1_w, d1_b, d2_w, d2_b, ln_g, ln_b):
    from concourse.bass_utils import run_bass_kernel_spmd

    queries = np.asarray(queries)
    B = queries.shape[0]
    vl1 = np.count_nonzero(np.asarray(mask_1), axis=1)
    vl2 = np.count_nonzero(np.asarray(mask_2), axis=1)
    k_pad = _pad128(vl1.max())
    q_pad = _pad128(vl2.max())
    nc = _get_program(k_pad, q_pad, B)
    in_maps = [
        make_in_map(b, k_pad, q_pad, queries, keys, values, mask_1, mask_2,
                    Wq, Wk, Wv, Wo, d1_w, d1_b, d2_w, d2_b, ln_g, ln_b)
        for b in range(B)
    ]
    res = run_bass_kernel_spmd(nc, in_maps, list(range(B)))
    return np.stack([res.results[b]["out"] for b in range(B)], axis=0)


# revision 19
# speedup vs baseline: 1.3158x; 1.3158x over previous
"""Trainium2 Bass kernel for a cross-attention transformer block.

Contract: kernel(**inputs) takes the FULL inputs (B=8 batch), shards
batch-wise across 8 NeuronCores (one batch element per core, SPMD, no
collectives), runs a Bass/Tile kernel, and returns the FULL output.

Per-core pipeline (everything stored feature-major, "X^T" [feat, tok],
so every linear is a single PE matmul pass with no transposes):
  Qp^T = (Wq/8)^T q^T   Kp^T = Wk^T k^T    (transposed-layout projections)
  Vp   = v^T-tiles as lhsT against Wv      (natural-layout projection)
  S^T  = Kp_h^T . Qp_h  per head (K=64, two heads row-packed in the PE)
  p    = exp(S^T + colNEG[k])              (ACT, per-partition bias; no max
                                            subtraction needed: |s| <~ 5)
  out^T= [Vp | 1] @ p                      (M=65: row 64 = softmax denom)
       + rank-1 corrections for masked query rows (exact, via K=1 matmuls)
  mha  = Wo-projection done twice (transposed for the FFN input, natural
         for the residual), FFN with fused relu+bias, residual + LayerNorm
         in natural layout, DMA out.

Numerics: fp32r matmuls (FP22 multiply / fp32 accumulate) ~1.5e-4 rel.
"""

import os
import sys

for _p in ("/opt/trn_rl_repo",):
    if _p not in sys.path:
        sys.path.insert(0, _p)

import numpy as np

import concourse.bacc as bacc
import concourse.tile as tile
from concourse import mybir

F32 = mybir.dt.float32
F32R = mybir.dt.float32r
AF = mybir.ActivationFunctionType
OP = mybir.AluOpType

D = 768
H = 12
HD = 64
DT = 6          # feature tiles of 128
L = 1024
NEG = -1000000.0
EPS = 1e-5

_CHUNKS = {
    256: [256], 384: [384], 512: [512], 640: [384, 256], 768: [512, 256],
    896: [512, 384], 1024: [512, 512],
}


def _chunks(width):
    out, off = [], 0
    for w in _CHUNKS[width]:
        out.append((off, w))
        off += w
    return out


def _pad128(n):
    return int(min(L, max(256, ((int(n) + 127) // 128) * 128)))


def build_program(k_pad, q_pad, n_cores, has_g=True, has_b=True, has_d2b=True):
    kt_n = k_pad // 128
    qch = _chunks(q_pad)
    kch = _chunks(k_pad)
    tail = L - q_pad  # rank-1b region width (may be 0)

    nc = bacc.Bacc("TRN2", target_bir_lowering=False, debug=False,
                   num_devices=n_cores)

    def din(name, shape, dt=F32):
        return nc.dram_tensor(name, shape, dt, kind="ExternalInput").ap()

    qT = din("qT", [D, q_pad])
    kT = din("kT", [D, k_pad])
    vT = din("vT", [D, k_pad])
    wq = din("wq", [D, D])
    wk = din("wk", [D, D])
    wv = din("wv", [D, D])
    wo = din("wo", [D, D])
    d1w = din("d1w", [D, D])
    d2w = din("d2w", [D, D])
    vauxm = din("vauxm", [128, DT])
    vauxa = din("vauxa", [128, DT])
    colneg = din("colneg", [128, kt_n])
    wvec = din("wvec", [1, q_pad])
    sigu = din("sigu", [1, H])
    d1b = din("d1b", [128, DT])
    d2b = din("d2b", [1, D])
    lng = din("lng", [1, D])
    lnb = din("lnb", [1, D])
    out = nc.dram_tensor("out", [L, D], F32, kind="ExternalOutput").ap()

    def dma_split(dst, src_ap, n):
        w = dst.shape[-1]
        step = (w + n - 1) // n
        for o in range(0, w, step):
            e = min(o + step, w)
            nc.sync.dma_start(out=dst[:, o:e], in_=src_ap[:, o:e])

    from contextlib import ExitStack
    with tile.TileContext(nc) as tc, ExitStack() as ctx:
        # ---------------- long-lived small tiles ----------------
        plong = ctx.enter_context(tc.tile_pool(name="plong", bufs=1))
        colneg_s = plong.tile([128, kt_n], F32, name="colneg_s")
        nc.sync.dma_start(out=colneg_s[:], in_=colneg[:, :])
        wvec_s = plong.tile([1, q_pad], F32R, name="wvec_s")
        nc.sync.dma_start(out=wvec_s[:], in_=wvec[:, :].bitcast(F32R))
        ones_s = plong.tile([1, 512], F32R, name="ones_s")
        ONE_BITS = 0x3F800000  # walrus rejects float32r memset; write bits
        nc.vector.memset(ones_s[:].bitcast(mybir.dt.uint32), ONE_BITS)
        vm65row = plong.tile([1, 65 * H], F32R, name="vm65row")
        va65row = plong.tile([1, 65 * H], F32R, name="va65row")
        vauxm_s = plong.tile([128, DT], F32R, name="vauxm_s")
        nc.sync.dma_start(out=vauxm_s[:], in_=vauxm[:, :].bitcast(F32R))
        vauxa_s = plong.tile([128, DT], F32R, name="vauxa_s")
        nc.sync.dma_start(out=vauxa_s[:], in_=vauxa[:, :].bitcast(F32R))
        gb = plong.tile([128, D], F32, name="gb") if has_g else None
        bb = plong.tile([128, D], F32, name="bb") if has_b else None
        d2bb = plong.tile([128, D], F32, name="d2bb") if has_d2b else None
        epst = plong.tile([128, 1], F32, name="epst")
        d1b_s = plong.tile([128, DT], F32, name="d1b_s")

        # attnorm^T lives from attention through the Wo projections
        sBC = ExitStack()
        pbc = sBC.enter_context(tc.tile_pool(name="pbc", bufs=1))
        attnorm = [pbc.tile([128, L], F32R, name=f"attnorm{j}")
                   for j in range(DT)]

        # ---------------- phase A+B scope ----------------
        sAB = ExitStack()
        pproj = sAB.enter_context(tc.tile_pool(name="pproj", bufs=1))
        Qp = [pproj.tile([128, q_pad], F32R, name=f"Qp{j}") for j in range(DT)]
        Kp = [pproj.tile([128, k_pad], F32R, name=f"Kp{j}") for j in range(DT)]
        Vm65 = [pproj.tile([128, 65 * H], F32R, name=f"Vm65_{k}")
                for k in range(kt_n)]
        # ---------------- phase A: projections ----------------
        sA = ExitStack()
        pin = sA.enter_context(tc.tile_pool(name="pin", bufs=1))
        pw = sA.enter_context(tc.tile_pool(name="pw", bufs=6))
        psA = sA.enter_context(tc.tile_pool(name="psA", bufs=1, space="PSUM"))

        qTs = [pin.tile([128, q_pad], F32R, name=f"qTs{t}") for t in range(DT)]
        kTs = [pin.tile([128, k_pad], F32R, name=f"kTs{t}") for t in range(DT)]
        vTs = [pin.tile([128, k_pad], F32R, name=f"vTs{t}") for t in range(DT)]
        wvs = [pin.tile([128, D], F32R, name=f"wvs{t}") for t in range(DT)]

        # Q/K projections in transposed layout, two dout tiles at a time.
        # Inputs are DMA'd just before their first use so the PE starts
        # as soon as the first weight slices land.
        for (wdram, xdram, xs, outts, chs) in (
                (wq, qT, qTs, Qp, qch), (wk, kT, kTs, Kp, kch)):
            for t in range(DT):
                r = slice(128 * t, 128 * t + 128)
                dma_split(xs[t], xdram[r, :].bitcast(F32R), 2)
            for jh in range(3):
                pss = {}
                for jj in range(2):
                    for (c0, cw) in chs:
                        pss[jj, c0] = psA.tile(
                            [128, cw], F32, tag="A", bufs=4,
                            name=f"psA_{id(wdram)%97}_{jh}_{jj}_{c0}",
                            padded_shape=[128, 768])
                for t in range(DT):
                    wt = pw.tile([128, 256], F32R, tag="wst",
                                 name=f"w_{id(wdram)%97}_{jh}_{t}")
                    nc.sync.dma_start(
                        out=wt[:],
                        in_=wdram[128 * t:128 * t + 128,
                                  256 * jh:256 * jh + 256].bitcast(F32R))
                    for jj in range(2):
                        for (c0, cw) in chs:
                            nc.tensor.matmul(
                                pss[jj, c0][:, :],
                                wt[:, 128 * jj:128 * jj + 128],
                                xs[t][:, c0:c0 + cw],
                                start=(t == 0), stop=(t == DT - 1))
                for jj in range(2):
                    j = 2 * jh + jj
                    for (c0, cw) in chs:
                        nc.scalar.copy(out=outts[j][:, c0:c0 + cw],
                                       in_=pss[jj, c0][:, :])

        for t in range(DT):
            r = slice(128 * t, 128 * t + 128)
            dma_split(vTs[t], vT[r, :].bitcast(F32R), 2)
            dma_split(wvs[t], wv[r, :].bitcast(F32R), 3)

        # V projection in natural layout -> Vm65 (65-stride gaps per head)
        for kt in range(kt_n):
            psv = psA.tile([128, D], F32, tag="A", bufs=4, name=f"psV{kt}",
                           padded_shape=[128, 768])
            for t in range(DT):
                for (n0, nw) in ((0, 512), (512, 256)):
                    nc.tensor.matmul(
                        psv[:, n0:n0 + nw],
                        vTs[t][:, 128 * kt:128 * kt + 128],
                        wvs[t][:, n0:n0 + nw],
                        start=(t == 0), stop=(t == DT - 1))
            src = psv[:, :].rearrange("p (h e) -> p h e", e=64)
            dst = Vm65[kt][:].rearrange("p (h e) -> p h e", e=65)[:, :, 0:64]
            nc.vector.tensor_copy(out=dst, in_=src)
            nc.vector.memset(
                Vm65[kt][:].bitcast(mybir.dt.uint32)
                .rearrange("p (h e) -> p h e", e=65)[:, :, 64:65], ONE_BITS)

        # aux sums: (sum of masked v rows) @ Wv and (sum of all v rows) @ Wv
        for (aux_s, rowt, scale) in ((vauxm_s, vm65row, 1.0),
                                     (vauxa_s, va65row, 1.0 / L)):
            psx = psA.tile([1, D], F32, tag="A", bufs=4,
                           name=f"psaux{scale!r}", padded_shape=[128, 768])
            for t in range(DT):
                for (n0, nw) in ((0, 512), (512, 256)):
                    nc.tensor.matmul(
                        psx[:, n0:n0 + nw], aux_s[:, t:t + 1],
                        wvs[t][:, n0:n0 + nw],
                        start=(t == 0), stop=(t == DT - 1))
            nc.scalar.mul(
                out=rowt[:].rearrange("p (h e) -> p h e", e=65)[:, :, 0:64],
                in_=psx[0:1, :].rearrange("p (h e) -> p h e", e=64),
                mul=scale)
        nc.sync.dma_start(
            out=vm65row[:].rearrange("p (h e) -> p h e", e=65)[:, :, 64:65],
            in_=sigu[:, :].bitcast(F32R).rearrange("p (h e) -> p h e", e=1))
        nc.vector.memset(
            va65row[:].bitcast(mybir.dt.uint32)
            .rearrange("p (h e) -> p h e", e=65)[:, :, 64:65], ONE_BITS)

        sA.close()

        # ---------------- phase B: attention ----------------
        ppexp = sAB.enter_context(tc.tile_pool(name="ppexp", bufs=4))
        pden = sAB.enter_context(tc.tile_pool(name="pden", bufs=1))
        psB = sAB.enter_context(tc.tile_pool(name="psB", bufs=1, space="PSUM"))
        # masked-query tail columns first: cheap rank-1 PE work that fills
        # the pipe while the first exp wave ramps on ACT
        if tail:
            for h in range(H):
                jt, po = h // 2, 64 * (h % 2)
                hs = slice(65 * h, 65 * h + 65)
                ao2 = psB.tile([65, tail], F32, tag="ao", bufs=4,
                               name=f"ao2_{h}", padded_shape=[65, 512])
                nc.tensor.matmul(ao2[:, :], va65row[0:1, hs],
                                 ones_s[0:1, 0:tail], start=True, stop=True)
                nc.vector.tensor_copy(out=attnorm[jt][po:po + 64, q_pad:L],
                                      in_=ao2[0:64, :])

        # head-pair outer: the two heads of a pair occupy PE row strips
        # 0-63 / 64-127, and their score matmuls are emitted back-to-back
        # so the PE runs them concurrently (K=64 row packing)
        for hp in range(DT):
            aos = {}
            for hx in (0, 1):
                for (q0, qw) in qch:
                    aos[hx, q0] = psB.tile(
                        [65, qw], F32, tag="ao", bufs=4,
                        name=f"ao{hp}_{hx}_{q0}", padded_shape=[65, 512])
            for kt in range(kt_n):
                for (q0, qw) in qch:
                    ps_pair = []
                    for hx in (0, 1):
                        po = 64 * hx
                        sc = psB.tile([128, qw], F32, tag="sc", bufs=2,
                                      name=f"sc{hp}_{hx}_{kt}_{q0}",
                                      padded_shape=[128, 512])
                        nc.tensor.matmul(
                            sc[:, :],
                            Kp[hp][po:po + 64, 128 * kt:128 * kt + 128],
                            Qp[hp][po:po + 64, q0:q0 + qw],
                            start=True, stop=True)
                        ps_pair.append(sc)
                    for hx in (0, 1):
                        h = 2 * hp + hx
                        p = ppexp.tile([128, qw], F32R, tag="p", bufs=6,
                                       name=f"p{h}_{kt}_{q0}",
                                       padded_shape=[128, 512])
                        nc.scalar.activation(out=p[:], in_=ps_pair[hx][:, :],
                                             func=AF.Exp,
                                             bias=colneg_s[:, kt:kt + 1],
                                             scale=1.0)
                        nc.tensor.matmul(
                            aos[hx, q0][:, :],
                            Vm65[kt][:, 65 * h:65 * h + 65], p[:, :],
                            start=(kt == 0), stop=False)
            for hx in (0, 1):
                h = 2 * hp + hx
                po = 64 * hx
                hs = slice(65 * h, 65 * h + 65)
                for (q0, qw) in qch:
                    ao = aos[hx, q0]
                    nc.tensor.matmul(ao[:, :], vm65row[0:1, hs],
                                     wvec_s[0:1, q0:q0 + qw],
                                     start=False, stop=True)
                    rc = pden.tile([1, qw], F32R, tag="rc", bufs=4,
                                   name=f"rc{h}_{q0}", padded_shape=[1, 512])
                    with nc.allow_low_precision(
                            reason="f32r annotation; fp22 recip is ample"):
                        nc.vector.reciprocal(out=rc[:], in_=ao[64:65, :])
                    rbp = psB.tile([64, qw], F32, tag="rb", bufs=2,
                                   name=f"rbp{h}_{q0}", padded_shape=[64, 512])
                    nc.tensor.matmul(rbp[:, :], ones_s[0:1, 0:64], rc[:],
                                     start=True, stop=True)
                    rbs = pden.tile([64, qw], F32, tag="rbs", bufs=3,
                                    name=f"rbs{h}_{q0}",
                                    padded_shape=[64, 512])
                    nc.vector.tensor_copy(out=rbs[:], in_=rbp[:, :])
                    nc.vector.tensor_tensor(
                        out=attnorm[hp][po:po + 64, q0:q0 + qw],
                        in0=ao[0:64, :], in1=rbs[:], op=OP.mult)

        sAB.close()

        # ---------------- phase C: Wo both layouts ----------------
        sCD = ExitStack()
        pcd = sCD.enter_context(tc.tile_pool(name="pcd", bufs=1, side="right"))
        mhaT = [pcd.tile([128, L], F32R, name=f"mhaT{j}") for j in range(DT)]
        mhaN = [pcd.tile([128, D], F32, name=f"mhaN{q}") for q in range(8)]

        if has_g:
            nc.sync.dma_start(out=gb[:], in_=lng.to_broadcast([128, D]))
        if has_b:
            nc.sync.dma_start(out=bb[:], in_=lnb.to_broadcast([128, D]))
        if has_d2b:
            nc.sync.dma_start(out=d2bb[:], in_=d2b.to_broadcast([128, D]))
        nc.vector.memset(epst[:], EPS)
        nc.sync.dma_start(out=d1b_s[:], in_=d1b[:, :])

        sC = ExitStack()
        pc_w = sC.enter_context(tc.tile_pool(name="pc_w", bufs=1))
        psC = sC.enter_context(tc.tile_pool(name="psC", bufs=1, space="PSUM"))
        wos = [pc_w.tile([128, D], F32R, name=f"wos{t}") for t in range(DT)]
        for t in range(DT):
            dma_split(wos[t], wo[128 * t:128 * t + 128, :].bitcast(F32R), 3)
        for q0 in (0, 512):
            for j in range(DT):
                ps = psC.tile([128, 512], F32, tag="C", bufs=3,
                              name=f"psT{j}_{q0}", padded_shape=[128, 768])
                for t in range(DT):
                    nc.tensor.matmul(ps[:, :],
                                     wos[t][:, 128 * j:128 * j + 128],
                                     attnorm[t][:, q0:q0 + 512],
                                     start=(t == 0), stop=(t == DT - 1))
                nc.vector.tensor_copy(out=mhaT[j][:, q0:q0 + 512],
                                      in_=ps[:, :])
            for qi in range(q0 // 128, q0 // 128 + 4):
                ps = psC.tile([128, D], F32, tag="C", bufs=3, name=f"psN{qi}",
                              padded_shape=[128, 768])
                for (n0, nw) in ((0, 512), (512, 256)):
                    for t in range(DT):
                        nc.tensor.matmul(ps[:, n0:n0 + nw],
                                         attnorm[t][:, 128 * qi:128 * qi + 128],
                                         wos[t][:, n0:n0 + nw],
                                         start=(t == 0), stop=(t == DT - 1))
                # residual input: mha (natural) + d2 bias, for x = ffn + .
                if has_d2b:
                    nc.vector.tensor_tensor(out=mhaN[qi][:], in0=ps[:, :],
                                            in1=d2bb[:], op=OP.add)
                else:
                    nc.vector.tensor_copy(out=mhaN[qi][:], in_=ps[:, :])
        sC.close()
        sBC.close()

        # ---------------- phase D: FFN + residual + LayerNorm ----------------
        sD = ExitStack()
        pd_w = sD.enter_context(tc.tile_pool(name="pd_w", bufs=1, side="right"))
        pdx = sD.enter_context(tc.tile_pool(name="pdx", bufs=1, side="right"))
        psmall = sD.enter_context(
            tc.tile_pool(name="psmall", bufs=8, side="right"))
        psD = sD.enter_context(tc.tile_pool(name="psD", bufs=1, space="PSUM"))

        reluT = [pdx.tile([128, L], F32R, name=f"reluT{j}") for j in range(DT)]
        d2s = [pd_w.tile([128, D], F32R, name=f"d2s{t}") for t in range(DT)]
        d1s = [pd_w.tile([128, D], F32R, name=f"d1s{t}") for t in range(DT)]
        for t in range(DT):
            dma_split(d2s[t], d2w[128 * t:128 * t + 128, :].bitcast(F32R), 3)
            dma_split(d1s[t], d1w[128 * t:128 * t + 128, :].bitcast(F32R), 3)

        inv_d = 1.0 / D

        def emit_ffn_ln(qi):
            ps = psD.tile([128, D], F32, tag="D", bufs=3, name=f"psff{qi}",
                          padded_shape=[128, 768])
            for (n0, nw) in ((0, 512), (512, 256)):
                for t in range(DT):
                    nc.tensor.matmul(ps[:, n0:n0 + nw],
                                     reluT[t][:, 128 * qi:128 * qi + 128],
                                     d2s[t][:, n0:n0 + nw],
                                     start=(t == 0), stop=(t == DT - 1))
            x = pdx.tile([128, D], F32, tag="x", bufs=2, name=f"x{qi}")
            xsum = psmall.tile([128, 1], F32, tag="s1", name=f"xsum{qi}")
            # fused: x = ffn + mha, xsum = row-sum(x), one DVE pass
            nc.vector.scalar_tensor_tensor(out=x[:], in0=ps[:, :], scalar=0.0,
                                           in1=mhaN[qi][:], op0=OP.bypass,
                                           op1=OP.add, accum_out=xsum[:])
            scr = pdx.tile([128, D], F32, tag="scr", bufs=2, name=f"scr{qi}")
            xsq = psmall.tile([128, 1], F32, tag="s2", name=f"xsq{qi}")
            nc.scalar.activation(out=scr[:], in_=x[:], func=AF.Square,
                                 accum_out=xsq[:])
            mu = psmall.tile([128, 1], F32, tag="s3", name=f"mu{qi}")
            nc.vector.tensor_scalar_mul(out=mu[:], in0=xsum[:], scalar1=inv_d)
            var = psmall.tile([128, 1], F32, tag="s4", name=f"var{qi}")
            # var = xsq/D - mu^2  ==  (xsq*1/D) - mu*mu
            mu2 = psmall.tile([128, 1], F32, tag="s5", name=f"mu2{qi}")
            nc.vector.tensor_tensor(out=mu2[:], in0=mu[:], in1=mu[:],
                                    op=OP.mult)
            nc.vector.scalar_tensor_tensor(out=var[:], in0=xsq[:],
                                           scalar=inv_d, in1=mu2[:],
                                           op0=OP.mult, op1=OP.subtract)
            std = psmall.tile([128, 1], F32, tag="s6", name=f"std{qi}")
            nc.scalar.activation(out=std[:], in_=var[:], func=AF.Sqrt,
                                 bias=epst[:], scale=1.0)
            rstd = psmall.tile([128, 1], F32, tag="s7", name=f"rstd{qi}")
            nc.vector.reciprocal(out=rstd[:], in_=std[:])
            nmb = psmall.tile([128, 1], F32, tag="s8", name=f"nmb{qi}")
            nc.vector.scalar_tensor_tensor(out=nmb[:], in0=mu[:], scalar=-1.0,
                                           in1=rstd[:], op0=OP.mult,
                                           op1=OP.mult)
            # xn = x*rstd + (-mu*rstd); optional *g (DVE) and +b (GpSimd)
            cur = scr
            nc.vector.tensor_scalar(out=cur[:], in0=x[:], scalar1=rstd[:],
                                    scalar2=nmb[:], op0=OP.mult, op1=OP.add)
            if has_g:
                nc.vector.tensor_tensor(out=x[:], in0=cur[:], in1=gb[:],
                                        op=OP.mult)
                cur = x
            if has_b:
                xo = pdx.tile([128, D], F32, tag="xo", bufs=2, name=f"xo{qi}")
                if qi >= 6:
                    nc.vector.tensor_tensor(out=xo[:], in0=cur[:], in1=bb[:],
                                            op=OP.add)
                else:
                    nc.gpsimd.tensor_tensor(out=xo[:], in0=cur[:], in1=bb[:],
                                            op=OP.add)
                cur = xo
            nc.sync.dma_start(out=out[128 * qi:128 * qi + 128, :],
                              in_=cur[:])

        for q0 in (0, 512):
            for j in range(DT):
                ps = psD.tile([128, 512], F32, tag="D", bufs=3,
                              name=f"psd1_{j}_{q0}", padded_shape=[128, 768])
                for t in range(DT):
                    nc.tensor.matmul(ps[:, :],
                                     d1s[t][:, 128 * j:128 * j + 128],
                                     mhaT[t][:, q0:q0 + 512],
                                     start=(t == 0), stop=(t == DT - 1))
                nc.scalar.activation(out=reluT[j][:, q0:q0 + 512],
                                     in_=ps[:, :], func=AF.Relu,
                                     bias=d1b_s[:, j:j + 1], scale=1.0)
            for qi in range(q0 // 128, q0 // 128 + 4):
                emit_ffn_ln(qi)
        sD.close()
        sCD.close()

    nc.compile()
    return nc


_PROGRAM_CACHE = {}


def _get_program(k_pad, q_pad, n_cores, has_g, has_b, has_d2b):
    key = (k_pad, q_pad, n_cores, has_g, has_b, has_d2b)
    if key not in _PROGRAM_CACHE:
        _PROGRAM_CACHE[key] = build_program(k_pad, q_pad, n_cores,
                                            has_g, has_b, has_d2b)
    return _PROGRAM_CACHE[key]


def make_in_map(b, k_pad, q_pad, queries, keys, values, mask_1, mask_2,
                Wq, Wk, Wv, Wo, d1_w, d1_b, d2_w, d2_b, ln_g, ln_b):
    kt_n = k_pad // 128
    f32 = np.float32
    vl1 = int(np.count_nonzero(mask_1[b]))
    vl2 = int(np.count_nonzero(mask_2[b]))
    row01 = (np.arange(L) < vl2).astype(f32)
    qmask = np.asarray(queries[b], f32) * row01[:, None]
    col01 = (np.arange(L) < vl1)
    cn = np.where(col01, 0.0, NEG).astype(f32)[:k_pad]
    vb = np.asarray(values[b], f32)
    vinmasked = vb[vl1:, :].sum(0, dtype=np.float64).astype(f32)
    vinall = vb.sum(0, dtype=np.float64).astype(f32)
    return {
        "qT": np.ascontiguousarray(qmask.T[:, :q_pad]),
        "kT": np.ascontiguousarray(np.asarray(keys[b], f32).T[:, :k_pad]),
        "vT": np.ascontiguousarray(vb.T[:, :k_pad]),
        "wq": np.ascontiguousarray(np.asarray(Wq, f32) * 0.125),
        "wk": np.ascontiguousarray(np.asarray(Wk, f32)),
        "wv": np.ascontiguousarray(np.asarray(Wv, f32)),
        "wo": np.ascontiguousarray(np.asarray(Wo, f32)),
        "d1w": np.ascontiguousarray(np.asarray(d1_w, f32)),
        "d2w": np.ascontiguousarray(np.asarray(d2_w, f32)),
        "vauxm": np.ascontiguousarray(vinmasked.reshape(DT, 128).T),
        "vauxa": np.ascontiguousarray(vinall.reshape(DT, 128).T),
        "colneg": np.ascontiguousarray(cn.reshape(kt_n, 128).T),
        "wvec": np.ascontiguousarray((1.0 - row01)[None, :q_pad]),
        "sigu": np.full((1, H), float(L - vl1), f32),
        "d1b": np.ascontiguousarray(np.asarray(d1_b, f32).reshape(DT, 128).T),
        "d2b": np.ascontiguousarray(np.asarray(d2_b, f32)[None, :]),
        "lng": np.ascontiguousarray(np.asarray(ln_g, f32)[None, :]),
        "lnb": np.ascontiguousarray(np.asarray(ln_b, f32)[None, :]),
    }


def kernel(queries, keys, values, mask_1, mask_2,
           Wq, Wk, Wv, Wo, d1_w, d1_b, d2_w, d2_b, ln_g, ln_b):
    from concourse.bass_utils import run_bass_kernel_spmd

    queries = np.asarray(queries)
    B = queries.shape[0]
    vl1 = np.count_nonzero(np.asarray(mask_1), axis=1)
    vl2 = np.count_nonzero(np.asarray(mask_2), axis=1)
    k_pad = _pad128(vl1.max())
    q_pad = _pad128(vl2.max())
    has_g = not np.all(np.asarray(ln_g) == 1.0)
    has_b = bool(np.any(np.asarray(ln_b)))
    has_d2b = bool(np.any(np.asarray(d2_b)))
    nc = _get_program(k_pad, q_pad, B, has_g, has_b, has_d2b)
    in_maps = [
        make_in_map(b, k_pad, q_pad, queries, keys, values, mask_1, mask_2,
                    Wq, Wk, Wv, Wo, d1_w, d1_b, d2_w, d2_b, ln_g, ln_b)
        for b in range(B)
    ]
    res = run_bass_kernel_spmd(nc, in_maps, list(range(B)))
    return np.stack([res.results[b]["out"] for b in range(B)], axis=0)


# revision 27
# speedup vs baseline: 1.4080x; 1.0701x over previous
"""Trainium2 Bass kernel for a cross-attention transformer block.

Contract: kernel(**inputs) takes the FULL inputs (B=8 batch), shards
batch-wise across 8 NeuronCores (one batch element per core, SPMD, no
collectives), runs a Bass/Tile kernel, and returns the FULL output.

Per-core pipeline (everything stored feature-major, "X^T" [feat, tok],
so every linear is a single PE matmul pass with no transposes):
  Qp^T = (Wq/8)^T q^T   Kp^T = Wk^T k^T    (transposed-layout projections)
  Vp   = v^T-tiles as lhsT against Wv      (natural-layout projection)
  S^T  = Kp_h^T . Qp_h  per head (K=64, two heads row-packed in the PE)
  p    = exp(S^T + colNEG[k])              (ACT, per-partition bias; no max
                                            subtraction needed: |s| <~ 5)
  out^T= [Vp | 1] @ p                      (M=65: row 64 = softmax denom)
       + rank-1 corrections for masked query rows (exact, via K=1 matmuls)
  mha  = Wo-projection done twice (transposed for the FFN input, natural
         for the residual), FFN with fused relu+bias, residual + LayerNorm
         in natural layout, DMA out.

Numerics: fp32r matmuls (FP22 multiply / fp32 accumulate) ~1.5e-4 rel.
"""

import os
import sys

for _p in ("/opt/trn_rl_repo",):
    if _p not in sys.path:
        sys.path.insert(0, _p)

import numpy as np

import concourse.bacc as bacc
import concourse.tile as tile
from concourse import mybir

F32 = mybir.dt.float32
F32R = mybir.dt.float32r
AF = mybir.ActivationFunctionType
OP = mybir.AluOpType

D = 768
H = 12
HD = 64
DT = 6          # feature tiles of 128
L = 1024
NEG = -1000000.0
EPS = 1e-5

_CHUNKS = {
    256: [256], 384: [384], 512: [512], 640: [384, 256], 768: [512, 256],
    896: [512, 384], 1024: [512, 512],
}


def _chunks(width):
    out, off = [], 0
    for w in _CHUNKS[width]:
        out.append((off, w))
        off += w
    return out


def _pad128(n):
    return int(min(L, max(256, ((int(n) + 127) // 128) * 128)))


def build_program(k_pad, q_pad, n_cores, has_g=True, has_b=True, has_d2b=True):
    kt_n = k_pad // 128
    qch = _chunks(q_pad)
    kch = _chunks(k_pad)
    tail = L - q_pad  # rank-1b region width (may be 0)

    nc = bacc.Bacc("TRN2", target_bir_lowering=False, debug=False,
                   num_devices=n_cores)

    def din(name, shape, dt=F32):
        return nc.dram_tensor(name, shape, dt, kind="ExternalInput").ap()

    qT = din("qT", [D, q_pad])
    kT = din("kT", [D, k_pad])
    vT = din("vT", [D, k_pad])
    wq = din("wq", [D, D])
    wk = din("wk", [D, D])
    wv = din("wv", [D, D])
    wo = din("wo", [D, D])
    d1w = din("d1w", [D, D])
    d2w = din("d2w", [D, D])
    vauxm = din("vauxm", [128, DT])
    vauxa = din("vauxa", [128, DT])
    colneg = din("colneg", [128, kt_n])
    wvec = din("wvec", [1, q_pad])
    sigu = din("sigu", [1, H])
    d1b = din("d1b", [128, DT])
    d2b = din("d2b", [1, D])
    lng = din("lng", [1, D])
    lnb = din("lnb", [1, D])
    out = nc.dram_tensor("out", [L, D], F32, kind="ExternalOutput").ap()

    def dma_split(dst, src_ap, n):
        w = dst.shape[-1]
        step = (w + n - 1) // n
        for o in range(0, w, step):
            e = min(o + step, w)
            nc.sync.dma_start(out=dst[:, o:e], in_=src_ap[:, o:e])

    from contextlib import ExitStack
    with tile.TileContext(nc) as tc, ExitStack() as ctx:
        # ---------------- long-lived small tiles ----------------
        plong = ctx.enter_context(tc.tile_pool(name="plong", bufs=1))
        colneg_s = plong.tile([128, kt_n], F32, name="colneg_s")
        wvec_s = plong.tile([1, q_pad], F32R, name="wvec_s")
        ones_s = plong.tile([1, 512], F32R, name="ones_s")
        ONE_BITS = 0x3F800000  # walrus rejects float32r memset; write bits
        nc.vector.memset(ones_s[:].bitcast(mybir.dt.uint32), ONE_BITS)
        vm65row = plong.tile([1, 65 * H], F32R, name="vm65row")
        va65row = plong.tile([1, 65 * H], F32R, name="va65row")
        vauxm_s = plong.tile([128, DT], F32R, name="vauxm_s")
        vauxa_s = plong.tile([128, DT], F32R, name="vauxa_s")
        gb = plong.tile([128, D], F32, name="gb") if has_g else None
        bb = plong.tile([128, D], F32, name="bb") if has_b else None
        d2bb = plong.tile([128, D], F32, name="d2bb") if has_d2b else None
        epst = plong.tile([128, 1], F32, name="epst")
        d1b_s = plong.tile([128, DT], F32, name="d1b_s")

        # attnorm^T lives from attention through the Wo projections
        sBC = ExitStack()
        pbc = sBC.enter_context(tc.tile_pool(name="pbc", bufs=1))
        attnorm = [pbc.tile([128, L], F32R, name=f"attnorm{j}")
                   for j in range(DT)]

        # ---------------- phase A+B scope ----------------
        sAB = ExitStack()
        pproj = sAB.enter_context(tc.tile_pool(name="pproj", bufs=1))
        Qp = [pproj.tile([128, q_pad], F32R, name=f"Qp{j}") for j in range(DT)]
        Kp = [pproj.tile([128, k_pad], F32R, name=f"Kp{j}") for j in range(DT)]
        Vm65 = [pproj.tile([128, 65 * H], F32R, name=f"Vm65_{k}")
                for k in range(kt_n)]
        # ---------------- phase A: projections ----------------
        sA = ExitStack()
        pin = sA.enter_context(tc.tile_pool(name="pin", bufs=1))
        pw = sA.enter_context(tc.tile_pool(name="pw", bufs=6))
        psA = sA.enter_context(tc.tile_pool(name="psA", bufs=1, space="PSUM"))

        qTs = [pin.tile([128, q_pad], F32R, name=f"qTs{t}") for t in range(DT)]
        kTs = [pin.tile([128, k_pad], F32R, name=f"kTs{t}") for t in range(DT)]
        vTs = [pin.tile([128, k_pad], F32R, name=f"vTs{t}") for t in range(DT)]
        wvs = [pin.tile([128, D], F32R, name=f"wvs{t}") for t in range(DT)]

        # Q/K projections in transposed layout, two dout tiles at a time.
        # Inputs are DMA'd just before their first use so the PE starts
        # as soon as the first weight slices land.
        for (wdram, xdram, xs, outts, chs) in (
                (wq, qT, qTs, Qp, qch), (wk, kT, kTs, Kp, kch)):
            for t in range(DT):
                r = slice(128 * t, 128 * t + 128)
                dma_split(xs[t], xdram[r, :].bitcast(F32R), 2)
            for jh in range(3):
                pss = {}
                for jj in range(2):
                    for (c0, cw) in chs:
                        pss[jj, c0] = psA.tile(
                            [128, cw], F32, tag="A", bufs=4,
                            name=f"psA_{id(wdram)%97}_{jh}_{jj}_{c0}",
                            padded_shape=[128, 768])
                for t in range(DT):
                    wt = pw.tile([128, 256], F32R, tag="wst",
                                 name=f"w_{id(wdram)%97}_{jh}_{t}")
                    nc.sync.dma_start(
                        out=wt[:],
                        in_=wdram[128 * t:128 * t + 128,
                                  256 * jh:256 * jh + 256].bitcast(F32R))
                    for jj in range(2):
                        for (c0, cw) in chs:
                            nc.tensor.matmul(
                                pss[jj, c0][:, :],
                                wt[:, 128 * jj:128 * jj + 128],
                                xs[t][:, c0:c0 + cw],
                                start=(t == 0), stop=(t == DT - 1))
                for jj in range(2):
                    j = 2 * jh + jj
                    for (c0, cw) in chs:
                        nc.scalar.copy(out=outts[j][:, c0:c0 + cw],
                                       in_=pss[jj, c0][:, :])

        nc.sync.dma_start(out=colneg_s[:], in_=colneg[:, :])
        nc.sync.dma_start(out=wvec_s[:], in_=wvec[:, :].bitcast(F32R))
        nc.sync.dma_start(out=vauxm_s[:], in_=vauxm[:, :].bitcast(F32R))
        nc.sync.dma_start(out=vauxa_s[:], in_=vauxa[:, :].bitcast(F32R))
        for t in range(DT):
            r = slice(128 * t, 128 * t + 128)
            dma_split(vTs[t], vT[r, :].bitcast(F32R), 2)
            dma_split(wvs[t], wv[r, :].bitcast(F32R), 3)

        # V projection in natural layout -> Vm65 (65-stride gaps per head)
        for kt in range(kt_n):
            psv = psA.tile([128, D], F32, tag="A", bufs=4, name=f"psV{kt}",
                           padded_shape=[128, 768])
            for t in range(DT):
                for (n0, nw) in ((0, 512), (512, 256)):
                    nc.tensor.matmul(
                        psv[:, n0:n0 + nw],
                        vTs[t][:, 128 * kt:128 * kt + 128],
                        wvs[t][:, n0:n0 + nw],
                        start=(t == 0), stop=(t == DT - 1))
            src = psv[:, :].rearrange("p (h e) -> p h e", e=64)
            dst = Vm65[kt][:].rearrange("p (h e) -> p h e", e=65)[:, :, 0:64]
            nc.vector.tensor_copy(out=dst, in_=src)
            nc.vector.memset(
                Vm65[kt][:].bitcast(mybir.dt.uint32)
                .rearrange("p (h e) -> p h e", e=65)[:, :, 64:65], ONE_BITS)

        # aux sums: (sum of masked v rows) @ Wv and (sum of all v rows) @ Wv
        for (aux_s, rowt, scale) in ((vauxm_s, vm65row, 1.0),
                                     (vauxa_s, va65row, 1.0 / L)):
            psx = psA.tile([1, D], F32, tag="A", bufs=4,
                           name=f"psaux{scale!r}", padded_shape=[128, 768])
            for t in range(DT):
                for (n0, nw) in ((0, 512), (512, 256)):
                    nc.tensor.matmul(
                        psx[:, n0:n0 + nw], aux_s[:, t:t + 1],
                        wvs[t][:, n0:n0 + nw],
                        start=(t == 0), stop=(t == DT - 1))
            nc.scalar.mul(
                out=rowt[:].rearrange("p (h e) -> p h e", e=65)[:, :, 0:64],
                in_=psx[0:1, :].rearrange("p (h e) -> p h e", e=64),
                mul=scale)
        nc.sync.dma_start(
            out=vm65row[:].rearrange("p (h e) -> p h e", e=65)[:, :, 64:65],
            in_=sigu[:, :].bitcast(F32R).rearrange("p (h e) -> p h e", e=1))
        nc.vector.memset(
            va65row[:].bitcast(mybir.dt.uint32)
            .rearrange("p (h e) -> p h e", e=65)[:, :, 64:65], ONE_BITS)

        sA.close()

        # ---------------- phase B: attention ----------------
        ppexp = sAB.enter_context(tc.tile_pool(name="ppexp", bufs=4))
        pden = sAB.enter_context(tc.tile_pool(name="pden", bufs=1))
        psB = sAB.enter_context(tc.tile_pool(name="psB", bufs=1, space="PSUM"))
        # masked-query tail columns first: cheap rank-1 PE work that fills
        # the pipe while the first exp wave ramps on ACT
        if tail:
            for h in range(H):
                jt, po = h // 2, 64 * (h % 2)
                hs = slice(65 * h, 65 * h + 65)
                ao2 = psB.tile([65, tail], F32, tag="ao", bufs=4,
                               name=f"ao2_{h}", padded_shape=[65, 512])
                nc.tensor.matmul(ao2[:, :], va65row[0:1, hs],
                                 ones_s[0:1, 0:tail], start=True, stop=True)
                nc.vector.tensor_copy(out=attnorm[jt][po:po + 64, q_pad:L],
                                      in_=ao2[0:64, :])

        # head-pair outer: the two heads of a pair occupy PE row strips
        # 0-63 / 64-127, and their score matmuls are emitted back-to-back
        # so the PE runs them concurrently (K=64 row packing)
        for hp in range(DT):
            aos = {}
            for hx in (0, 1):
                for (q0, qw) in qch:
                    aos[hx, q0] = psB.tile(
                        [65, qw], F32, tag="ao", bufs=4,
                        name=f"ao{hp}_{hx}_{q0}", padded_shape=[65, 512])
            for kt in range(kt_n):
                for (q0, qw) in qch:
                    ps_pair = []
                    for hx in (0, 1):
                        po = 64 * hx
                        sc = psB.tile([128, qw], F32, tag="sc", bufs=3,
                                      name=f"sc{hp}_{hx}_{kt}_{q0}",
                                      padded_shape=[128, 512])
                        nc.tensor.matmul(
                            sc[:, :],
                            Kp[hp][po:po + 64, 128 * kt:128 * kt + 128],
                            Qp[hp][po:po + 64, q0:q0 + qw],
                            start=True, stop=True)
                        ps_pair.append(sc)
                    for hx in (0, 1):
                        h = 2 * hp + hx
                        p = ppexp.tile([128, qw], F32R, tag="p", bufs=6,
                                       name=f"p{h}_{kt}_{q0}",
                                       padded_shape=[128, 512])
                        nc.scalar.activation(out=p[:], in_=ps_pair[hx][:, :],
                                             func=AF.Exp,
                                             bias=colneg_s[:, kt:kt + 1],
                                             scale=1.0)
                        nc.tensor.matmul(
                            aos[hx, q0][:, :],
                            Vm65[kt][:, 65 * h:65 * h + 65], p[:, :],
                            start=(kt == 0), stop=False)
            for hx in (0, 1):
                h = 2 * hp + hx
                po = 64 * hx
                hs = slice(65 * h, 65 * h + 65)
                for (q0, qw) in qch:
                    ao = aos[hx, q0]
                    nc.tensor.matmul(ao[:, :], vm65row[0:1, hs],
                                     wvec_s[0:1, q0:q0 + qw],
                                     start=False, stop=True)
                    rc = pden.tile([1, qw], F32R, tag="rc", bufs=4,
                                   name=f"rc{h}_{q0}", padded_shape=[1, 512])
                    with nc.allow_low_precision(
                            reason="f32r annotation; fp22 recip is ample"):
                        nc.vector.reciprocal(out=rc[:], in_=ao[64:65, :])
                    rbp = psB.tile([64, qw], F32, tag="rb", bufs=1,
                                   name=f"rbp{h}_{q0}", padded_shape=[64, 512])
                    nc.tensor.matmul(rbp[:, :], ones_s[0:1, 0:64], rc[:],
                                     start=True, stop=True)
                    rbs = pden.tile([64, qw], F32, tag="rbs", bufs=3,
                                    name=f"rbs{h}_{q0}",
                                    padded_shape=[64, 512])
                    nc.vector.tensor_copy(out=rbs[:], in_=rbp[:, :])
                    nc.vector.tensor_tensor(
                        out=attnorm[hp][po:po + 64, q0:q0 + qw],
                        in0=ao[0:64, :], in1=rbs[:], op=OP.mult)

        sAB.close()

        # ---------------- phase C: Wo both layouts ----------------
        sCD = ExitStack()
        pcd = sCD.enter_context(tc.tile_pool(name="pcd", bufs=1, side="right"))
        mhaT = [pcd.tile([128, L], F32R, name=f"mhaT{j}") for j in range(DT)]
        mhaN = [pcd.tile([128, D], F32, name=f"mhaN{q}") for q in range(8)]

        if has_g:
            nc.sync.dma_start(out=gb[:], in_=lng.to_broadcast([128, D]))
        if has_b:
            nc.sync.dma_start(out=bb[:], in_=lnb.to_broadcast([128, D]))
        if has_d2b:
            nc.sync.dma_start(out=d2bb[:], in_=d2b.to_broadcast([128, D]))
        nc.vector.memset(epst[:], EPS)
        nc.sync.dma_start(out=d1b_s[:], in_=d1b[:, :])

        sC = ExitStack()
        pc_w = sC.enter_context(tc.tile_pool(name="pc_w", bufs=1))
        psC = sC.enter_context(tc.tile_pool(name="psC", bufs=1, space="PSUM"))
        from concourse.masks import make_identity
        ident = pc_w.tile([128, 128], F32R, name="ident")
        nc.vector.memset(ident[:].bitcast(mybir.dt.uint32), 0)
        make_identity(nc, ident[:], nomemset=True)
        wos = [pc_w.tile([128, D], F32R, name=f"wos{t}") for t in range(DT)]
        for t in range(DT):
            dma_split(wos[t], wo[128 * t:128 * t + 128, :].bitcast(F32R), 3)
        for q0 in (0, 512):
            for j in range(DT):
                ps = psC.tile([128, 512], F32, tag="C", bufs=3,
                              name=f"psT{j}_{q0}", padded_shape=[128, 768])
                for t in range(DT):
                    nc.tensor.matmul(ps[:, :],
                                     wos[t][:, 128 * j:128 * j + 128],
                                     attnorm[t][:, q0:q0 + 512],
                                     start=(t == 0), stop=(t == DT - 1))
                nc.vector.tensor_copy(out=mhaT[j][:, q0:q0 + 512],
                                      in_=ps[:, :])
            for qi in range(q0 // 128, q0 // 128 + 4):
                # mha natural = blockwise PE transpose of mhaT
                for j in range(DT):
                    tp = psC.tile([128, 128], F32R, tag="tr", bufs=2,
                                  name=f"tp{qi}_{j}")
                    nc.tensor.transpose(
                        tp[:, :], mhaT[j][:, 128 * qi:128 * qi + 128],
                        ident[:])
                    dst = mhaN[qi][:, 128 * j:128 * j + 128]
                    if has_d2b:
                        nc.vector.tensor_tensor(out=dst, in0=tp[:, :],
                                                in1=d2bb[:, 128 * j:128 * j + 128],
                                                op=OP.add)
                    else:
                        nc.vector.tensor_copy(out=dst, in_=tp[:, :])
        sC.close()
        sBC.close()

        # ---------------- phase D: FFN + residual + LayerNorm ----------------
        sD = ExitStack()
        pd_w = sD.enter_context(tc.tile_pool(name="pd_w", bufs=1, side="right"))
        pdx = sD.enter_context(tc.tile_pool(name="pdx", bufs=1, side="right"))
        psmall = sD.enter_context(
            tc.tile_pool(name="psmall", bufs=8, side="right"))
        psD = sD.enter_context(tc.tile_pool(name="psD", bufs=1, space="PSUM"))

        reluT = [pdx.tile([128, L], F32R, name=f"reluT{j}") for j in range(DT)]
        d2s = [pd_w.tile([128, D], F32R, name=f"d2s{t}") for t in range(DT)]
        d1s = [pd_w.tile([128, D], F32R, name=f"d1s{t}") for t in range(DT)]
        for t in range(DT):
            dma_split(d2s[t], d2w[128 * t:128 * t + 128, :].bitcast(F32R), 3)
            dma_split(d1s[t], d1w[128 * t:128 * t + 128, :].bitcast(F32R), 3)

        inv_d = 1.0 / D

        def emit_ffn_ln(qi):
            ps = psD.tile([128, D], F32, tag="D", bufs=3, name=f"psff{qi}",
                          padded_shape=[128, 768])
            for (n0, nw) in ((0, 512), (512, 256)):
                for t in range(DT):
                    nc.tensor.matmul(ps[:, n0:n0 + nw],
                                     reluT[t][:, 128 * qi:128 * qi + 128],
                                     d2s[t][:, n0:n0 + nw],
                                     start=(t == 0), stop=(t == DT - 1))
            x = pdx.tile([128, D], F32, tag="x", bufs=2, name=f"x{qi}")
            xsum = psmall.tile([128, 1], F32, tag="s1", name=f"xsum{qi}")
            # fused: x = ffn + mha, xsum = row-sum(x), one DVE pass
            nc.vector.scalar_tensor_tensor(out=x[:], in0=ps[:, :], scalar=0.0,
                                           in1=mhaN[qi][:], op0=OP.bypass,
                                           op1=OP.add, accum_out=xsum[:])
            scr = pdx.tile([128, D], F32, tag="scr", bufs=2, name=f"scr{qi}")
            xsq = psmall.tile([128, 1], F32, tag="s2", name=f"xsq{qi}")
            nc.scalar.activation(out=scr[:], in_=x[:], func=AF.Square,
                                 accum_out=xsq[:])
            mu = psmall.tile([128, 1], F32, tag="s3", name=f"mu{qi}")
            nc.vector.tensor_scalar_mul(out=mu[:], in0=xsum[:], scalar1=inv_d)
            var = psmall.tile([128, 1], F32, tag="s4", name=f"var{qi}")
            # var = xsq/D - mu^2  ==  (xsq*1/D) - mu*mu
            mu2 = psmall.tile([128, 1], F32, tag="s5", name=f"mu2{qi}")
            nc.vector.tensor_tensor(out=mu2[:], in0=mu[:], in1=mu[:],
                                    op=OP.mult)
            nc.vector.scalar_tensor_tensor(out=var[:], in0=xsq[:],
                                           scalar=inv_d, in1=mu2[:],
                                           op0=OP.mult, op1=OP.subtract)
            std = psmall.tile([128, 1], F32, tag="s6", name=f"std{qi}")
            nc.scalar.activation(out=std[:], in_=var[:], func=AF.Sqrt,
                                 bias=epst[:], scale=1.0)
            rstd = psmall.tile([128, 1], F32, tag="s7", name=f"rstd{qi}")
            nc.vector.reciprocal(out=rstd[:], in_=std[:])
            nmb = psmall.tile([128, 1], F32, tag="s8", name=f"nmb{qi}")
            nc.vector.scalar_tensor_tensor(out=nmb[:], in0=mu[:], scalar=-1.0,
                                           in1=rstd[:], op0=OP.mult,
                                           op1=OP.mult)
            # xn = x*rstd + (-mu*rstd); optional *g (DVE) and +b (GpSimd)
            cur = scr
            nc.vector.tensor_scalar(out=cur[:], in0=x[:], scalar1=rstd[:],
                                    scalar2=nmb[:], op0=OP.mult, op1=OP.add)
            if has_g:
                nc.vector.tensor_tensor(out=x[:], in0=cur[:], in1=gb[:],
                                        op=OP.mult)
                cur = x
            if has_b:
                xo = pdx.tile([128, D], F32, tag="xo", bufs=2, name=f"xo{qi}")
                if qi >= 6:
                    nc.vector.tensor_tensor(out=xo[:], in0=cur[:], in1=bb[:],
                                            op=OP.add)
                else:
                    nc.gpsimd.tensor_tensor(out=xo[:], in0=cur[:], in1=bb[:],
                                            op=OP.add)
                cur = xo
            nc.sync.dma_start(out=out[128 * qi:128 * qi + 128, :],
                              in_=cur[:])

        for q0 in (0, 512):
            for j in range(DT):
                ps = psD.tile([128, 512], F32, tag="D", bufs=3,
                              name=f"psd1_{j}_{q0}", padded_shape=[128, 768])
                for t in range(DT):
                    nc.tensor.matmul(ps[:, :],
                                     d1s[t][:, 128 * j:128 * j + 128],
                                     mhaT[t][:, q0:q0 + 512],
                                     start=(t == 0), stop=(t == DT - 1))
                nc.scalar.activation(out=reluT[j][:, q0:q0 + 512],
                                     in_=ps[:, :], func=AF.Relu,
                                     bias=d1b_s[:, j:j + 1], scale=1.0)
            for qi in range(q0 // 128, q0 // 128 + 4):
                emit_ffn_ln(qi)
        sD.close()
        sCD.close()

    nc.compile()
    return nc


_PROGRAM_CACHE = {}


def _get_program(k_pad, q_pad, n_cores, has_g, has_b, has_d2b):
    key = (k_pad, q_pad, n_cores, has_g, has_b, has_d2b)
    if key not in _PROGRAM_CACHE:
        _PROGRAM_CACHE[key] = build_program(k_pad, q_pad, n_cores,
                                            has_g, has_b, has_d2b)
    return _PROGRAM_CACHE[key]


def make_in_map(b, k_pad, q_pad, queries, keys, values, mask_1, mask_2,
                Wq, Wk, Wv, Wo, d1_w, d1_b, d2_w, d2_b, ln_g, ln_b):
    kt_n = k_pad // 128
    f32 = np.float32
    vl1 = int(np.count_nonzero(mask_1[b]))
    vl2 = int(np.count_nonzero(mask_2[b]))
    row01 = (np.arange(L) < vl2).astype(f32)
    qmask = np.asarray(queries[b], f32) * row01[:, None]
    col01 = (np.arange(L) < vl1)
    cn = np.where(col01, 0.0, NEG).astype(f32)[:k_pad]
    vb = np.asarray(values[b], f32)
    vinmasked = vb[vl1:, :].sum(0, dtype=np.float64).astype(f32)
    vinall = vb.sum(0, dtype=np.float64).astype(f32)
    return {
        "qT": np.ascontiguousarray(qmask.T[:, :q_pad]),
        "kT": np.ascontiguousarray(np.asarray(keys[b], f32).T[:, :k_pad]),
        "vT": np.ascontiguousarray(vb.T[:, :k_pad]),
        "wq": np.ascontiguousarray(np.asarray(Wq, f32) * 0.125),
        "wk": np.ascontiguousarray(np.asarray(Wk, f32)),
        "wv": np.ascontiguousarray(np.asarray(Wv, f32)),
        "wo": np.ascontiguousarray(np.asarray(Wo, f32)),
        "d1w": np.ascontiguousarray(np.asarray(d1_w, f32)),
        "d2w": np.ascontiguousarray(np.asarray(d2_w, f32)),
        "vauxm": np.ascontiguousarray(vinmasked.reshape(DT, 128).T),
        "vauxa": np.ascontiguousarray(vinall.reshape(DT, 128).T),
        "colneg": np.ascontiguousarray(cn.reshape(kt_n, 128).T),
        "wvec": np.ascontiguousarray((1.0 - row01)[None, :q_pad]),
        "sigu": np.full((1, H), float(L - vl1), f32),
        "d1b": np.ascontiguousarray(np.asarray(d1_b, f32).reshape(DT, 128).T),
        "d2b": np.ascontiguousarray(np.asarray(d2_b, f32)[None, :]),
        "lng": np.ascontiguousarray(np.asarray(ln_g, f32)[None, :]),
        "lnb": np.ascontiguousarray(np.asarray(ln_b, f32)[None, :]),
    }


def kernel(queries, keys, values, mask_1, mask_2,
           Wq, Wk, Wv, Wo, d1_w, d1_b, d2_w, d2_b, ln_g, ln_b):
    from concourse.bass_utils import run_bass_kernel_spmd

    queries = np.asarray(queries)
    B = queries.shape[0]
    vl1 = np.count_nonzero(np.asarray(mask_1), axis=1)
    vl2 = np.count_nonzero(np.asarray(mask_2), axis=1)
    k_pad = _pad128(vl1.max())
    q_pad = _pad128(vl2.max())
    has_g = not np.all(np.asarray(ln_g) == 1.0)
    has_b = bool(np.any(np.asarray(ln_b)))
    has_d2b = bool(np.any(np.asarray(d2_b)))
    nc = _get_program(k_pad, q_pad, B, has_g, has_b, has_d2b)
    in_maps = [
        make_in_map(b, k_pad, q_pad, queries, keys, values, mask_1, mask_2,
                    Wq, Wk, Wv, Wo, d1_w, d1_b, d2_w, d2_b, ln_g, ln_b)
        for b in range(B)
    ]
    res = run_bass_kernel_spmd(nc, in_maps, list(range(B)))
    return np.stack([res.results[b]["out"] for b in range(B)], axis=0)


# revision 28
# speedup vs baseline: 1.4141x; 1.0044x over previous
"""Trainium2 Bass kernel for a cross-attention transformer block.

Contract: kernel(**inputs) takes the FULL inputs (B=8 batch), shards
batch-wise across 8 NeuronCores (one batch element per core, SPMD, no
collectives), runs a Bass/Tile kernel, and returns the FULL output.

Per-core pipeline (everything stored feature-major, "X^T" [feat, tok],
so every linear is a single PE matmul pass with no transposes):
  Qp^T = (Wq/8)^T q^T   Kp^T = Wk^T k^T    (transposed-layout projections)
  Vp   = v^T-tiles as lhsT against Wv      (natural-layout projection)
  S^T  = Kp_h^T . Qp_h  per head (K=64, two heads row-packed in the PE)
  p    = exp(S^T + colNEG[k])              (ACT, per-partition bias; no max
                                            subtraction needed: |s| <~ 5)
  out^T= [Vp | 1] @ p                      (M=65: row 64 = softmax denom)
       + rank-1 corrections for masked query rows (exact, via K=1 matmuls)
  mha  = Wo-projection done twice (transposed for the FFN input, natural
         for the residual), FFN with fused relu+bias, residual + LayerNorm
         in natural layout, DMA out.

Numerics: fp32r matmuls (FP22 multiply / fp32 accumulate) ~1.5e-4 rel.
"""

import os
import sys

for _p in ("/opt/trn_rl_repo",):
    if _p not in sys.path:
        sys.path.insert(0, _p)

import numpy as np

import concourse.bacc as bacc
import concourse.tile as tile
from concourse import mybir

F32 = mybir.dt.float32
F32R = mybir.dt.float32r
AF = mybir.ActivationFunctionType
OP = mybir.AluOpType

D = 768
H = 12
HD = 64
DT = 6          # feature tiles of 128
L = 1024
NEG = -1000000.0
EPS = 1e-5

_CHUNKS = {
    256: [256], 384: [384], 512: [512], 640: [384, 256], 768: [512, 256],
    896: [512, 384], 1024: [512, 512],
}


def _chunks(width):
    out, off = [], 0
    for w in _CHUNKS[width]:
        out.append((off, w))
        off += w
    return out


def _pad128(n):
    return int(min(L, max(256, ((int(n) + 127) // 128) * 128)))


def build_program(k_pad, q_pad, n_cores, has_g=True, has_b=True, has_d2b=True):
    kt_n = k_pad // 128
    qch = _chunks(q_pad)
    kch = _chunks(k_pad)
    tail = L - q_pad  # rank-1b region width (may be 0)

    nc = bacc.Bacc("TRN2", target_bir_lowering=False, debug=False,
                   num_devices=n_cores)

    def din(name, shape, dt=F32):
        return nc.dram_tensor(name, shape, dt, kind="ExternalInput").ap()

    qT = din("qT", [D, q_pad])
    kT = din("kT", [D, k_pad])
    vT = din("vT", [D, k_pad])
    wq = din("wq", [D, D])
    wk = din("wk", [D, D])
    wv = din("wv", [D, D])
    wo = din("wo", [D, D])
    d1w = din("d1w", [D, D])
    d2w = din("d2w", [D, D])
    vauxm = din("vauxm", [128, DT])
    vauxa = din("vauxa", [128, DT])
    colneg = din("colneg", [128, kt_n])
    wvec = din("wvec", [1, q_pad])
    sigu = din("sigu", [1, H])
    d1b = din("d1b", [128, DT])
    d2b = din("d2b", [1, D])
    lng = din("lng", [1, D])
    lnb = din("lnb", [1, D])
    out = nc.dram_tensor("out", [L, D], F32, kind="ExternalOutput").ap()

    def dma_split(dst, src_ap, n):
        w = dst.shape[-1]
        step = (w + n - 1) // n
        for o in range(0, w, step):
            e = min(o + step, w)
            nc.sync.dma_start(out=dst[:, o:e], in_=src_ap[:, o:e])

    from contextlib import ExitStack
    with tile.TileContext(nc) as tc, ExitStack() as ctx:
        # ---------------- long-lived small tiles ----------------
        plong = ctx.enter_context(tc.tile_pool(name="plong", bufs=1))
        colneg_s = plong.tile([128, kt_n], F32, name="colneg_s")
        wvec_s = plong.tile([1, q_pad], F32R, name="wvec_s")
        ones_s = plong.tile([1, 512], F32R, name="ones_s")
        ONE_BITS = 0x3F800000  # walrus rejects float32r memset; write bits
        nc.vector.memset(ones_s[:].bitcast(mybir.dt.uint32), ONE_BITS)
        vm65row = plong.tile([1, 65 * H], F32R, name="vm65row")
        va65row = plong.tile([1, 65 * H], F32R, name="va65row")
        vauxm_s = plong.tile([128, DT], F32R, name="vauxm_s")
        vauxa_s = plong.tile([128, DT], F32R, name="vauxa_s")
        gb = plong.tile([128, D], F32, name="gb") if has_g else None
        bb = plong.tile([128, D], F32, name="bb") if has_b else None
        d2bb = plong.tile([128, D], F32, name="d2bb") if has_d2b else None
        epst = plong.tile([128, 1], F32, name="epst")
        d1b_s = plong.tile([128, DT], F32, name="d1b_s")

        # attnorm^T lives from attention through the Wo projections
        sBC = ExitStack()
        pbc = sBC.enter_context(tc.tile_pool(name="pbc", bufs=1))
        attnorm = [pbc.tile([128, L], F32R, name=f"attnorm{j}")
                   for j in range(DT)]

        # ---------------- phase A+B scope ----------------
        sAB = ExitStack()
        pproj = sAB.enter_context(tc.tile_pool(name="pproj", bufs=1))
        Qp = [pproj.tile([128, q_pad], F32R, name=f"Qp{j}") for j in range(DT)]
        Kp = [pproj.tile([128, k_pad], F32R, name=f"Kp{j}") for j in range(DT)]
        Vm65 = [pproj.tile([128, 65 * H], F32R, name=f"Vm65_{k}")
                for k in range(kt_n)]
        # ---------------- phase A: projections ----------------
        sA = ExitStack()
        pin = sA.enter_context(tc.tile_pool(name="pin", bufs=1))
        pw = sA.enter_context(tc.tile_pool(name="pw", bufs=6))
        psA = sA.enter_context(tc.tile_pool(name="psA", bufs=1, space="PSUM"))

        qTs = [pin.tile([128, q_pad], F32R, name=f"qTs{t}") for t in range(DT)]
        kTs = [pin.tile([128, k_pad], F32R, name=f"kTs{t}") for t in range(DT)]
        vTs = [pin.tile([128, k_pad], F32R, name=f"vTs{t}") for t in range(DT)]
        wvs = [pin.tile([128, D], F32R, name=f"wvs{t}") for t in range(DT)]

        # Q/K projections in transposed layout, two dout tiles at a time.
        # Inputs are DMA'd just before their first use so the PE starts
        # as soon as the first weight slices land.
        for (wdram, xdram, xs, outts, chs) in (
                (wq, qT, qTs, Qp, qch), (wk, kT, kTs, Kp, kch)):
            for t in range(DT):
                r = slice(128 * t, 128 * t + 128)
                dma_split(xs[t], xdram[r, :].bitcast(F32R), 2)
            for jh in range(3):
                pss = {}
                for jj in range(2):
                    for (c0, cw) in chs:
                        pss[jj, c0] = psA.tile(
                            [128, cw], F32, tag="A", bufs=4,
                            name=f"psA_{id(wdram)%97}_{jh}_{jj}_{c0}",
                            padded_shape=[128, 768])
                for t in range(DT):
                    wt = pw.tile([128, 256], F32R, tag="wst",
                                 name=f"w_{id(wdram)%97}_{jh}_{t}")
                    nc.sync.dma_start(
                        out=wt[:],
                        in_=wdram[128 * t:128 * t + 128,
                                  256 * jh:256 * jh + 256].bitcast(F32R))
                    for jj in range(2):
                        for (c0, cw) in chs:
                            nc.tensor.matmul(
                                pss[jj, c0][:, :],
                                wt[:, 128 * jj:128 * jj + 128],
                                xs[t][:, c0:c0 + cw],
                                start=(t == 0), stop=(t == DT - 1))
                for jj in range(2):
                    j = 2 * jh + jj
                    for (c0, cw) in chs:
                        nc.scalar.copy(out=outts[j][:, c0:c0 + cw],
                                       in_=pss[jj, c0][:, :])

        nc.sync.dma_start(out=colneg_s[:], in_=colneg[:, :])
        nc.sync.dma_start(out=wvec_s[:], in_=wvec[:, :].bitcast(F32R))
        nc.sync.dma_start(out=vauxm_s[:], in_=vauxm[:, :].bitcast(F32R))
        nc.sync.dma_start(out=vauxa_s[:], in_=vauxa[:, :].bitcast(F32R))
        for t in range(DT):
            r = slice(128 * t, 128 * t + 128)
            dma_split(vTs[t], vT[r, :].bitcast(F32R), 2)
            dma_split(wvs[t], wv[r, :].bitcast(F32R), 3)

        # V projection in natural layout -> Vm65 (65-stride gaps per head)
        for kt in range(kt_n):
            psv = psA.tile([128, D], F32, tag="A", bufs=4, name=f"psV{kt}",
                           padded_shape=[128, 768])
            for t in range(DT):
                for (n0, nw) in ((0, 512), (512, 256)):
                    nc.tensor.matmul(
                        psv[:, n0:n0 + nw],
                        vTs[t][:, 128 * kt:128 * kt + 128],
                        wvs[t][:, n0:n0 + nw],
                        start=(t == 0), stop=(t == DT - 1))
            src = psv[:, :].rearrange("p (h e) -> p h e", e=64)
            dst = Vm65[kt][:].rearrange("p (h e) -> p h e", e=65)[:, :, 0:64]
            nc.vector.tensor_copy(out=dst, in_=src)
            nc.vector.memset(
                Vm65[kt][:].bitcast(mybir.dt.uint32)
                .rearrange("p (h e) -> p h e", e=65)[:, :, 64:65], ONE_BITS)

        # aux sums: (sum of masked v rows) @ Wv and (sum of all v rows) @ Wv
        for (aux_s, rowt, scale) in ((vauxm_s, vm65row, 1.0),
                                     (vauxa_s, va65row, 1.0 / L)):
            psx = psA.tile([1, D], F32, tag="A", bufs=4,
                           name=f"psaux{scale!r}", padded_shape=[128, 768])
            for t in range(DT):
                for (n0, nw) in ((0, 512), (512, 256)):
                    nc.tensor.matmul(
                        psx[:, n0:n0 + nw], aux_s[:, t:t + 1],
                        wvs[t][:, n0:n0 + nw],
                        start=(t == 0), stop=(t == DT - 1))
            nc.scalar.mul(
                out=rowt[:].rearrange("p (h e) -> p h e", e=65)[:, :, 0:64],
                in_=psx[0:1, :].rearrange("p (h e) -> p h e", e=64),
                mul=scale)
        nc.sync.dma_start(
            out=vm65row[:].rearrange("p (h e) -> p h e", e=65)[:, :, 64:65],
            in_=sigu[:, :].bitcast(F32R).rearrange("p (h e) -> p h e", e=1))
        nc.vector.memset(
            va65row[:].bitcast(mybir.dt.uint32)
            .rearrange("p (h e) -> p h e", e=65)[:, :, 64:65], ONE_BITS)

        sA.close()

        # ---------------- phase B: attention ----------------
        ppexp = sAB.enter_context(tc.tile_pool(name="ppexp", bufs=4))
        pden = sAB.enter_context(tc.tile_pool(name="pden", bufs=1))
        psB = sAB.enter_context(tc.tile_pool(name="psB", bufs=1, space="PSUM"))
        # masked-query tail columns first: cheap rank-1 PE work that fills
        # the pipe while the first exp wave ramps on ACT
        if tail:
            for h in range(H):
                jt, po = h // 2, 64 * (h % 2)
                hs = slice(65 * h, 65 * h + 65)
                ao2 = psB.tile([65, tail], F32, tag="ao", bufs=4,
                               name=f"ao2_{h}", padded_shape=[65, 512])
                nc.tensor.matmul(ao2[:, :], va65row[0:1, hs],
                                 ones_s[0:1, 0:tail], start=True, stop=True)
                nc.vector.tensor_copy(out=attnorm[jt][po:po + 64, q_pad:L],
                                      in_=ao2[0:64, :])

        # head-pair outer: the two heads of a pair occupy PE row strips
        # 0-63 / 64-127, and their score matmuls are emitted back-to-back
        # so the PE runs them concurrently (K=64 row packing)
        for hp in range(DT):
            aos = {}
            for hx in (0, 1):
                for (q0, qw) in qch:
                    aos[hx, q0] = psB.tile(
                        [65, qw], F32, tag="ao", bufs=4,
                        name=f"ao{hp}_{hx}_{q0}", padded_shape=[65, 512])
            for kt in range(kt_n):
                for (q0, qw) in qch:
                    ps_pair = []
                    for hx in (0, 1):
                        po = 64 * hx
                        sc = psB.tile([128, qw], F32, tag="sc", bufs=3,
                                      name=f"sc{hp}_{hx}_{kt}_{q0}",
                                      padded_shape=[128, 512])
                        nc.tensor.matmul(
                            sc[:, :],
                            Kp[hp][po:po + 64, 128 * kt:128 * kt + 128],
                            Qp[hp][po:po + 64, q0:q0 + qw],
                            start=True, stop=True)
                        ps_pair.append(sc)
                    for hx in (0, 1):
                        h = 2 * hp + hx
                        p = ppexp.tile([128, qw], F32R, tag="p", bufs=8,
                                       name=f"p{h}_{kt}_{q0}",
                                       padded_shape=[128, 512])
                        nc.scalar.activation(out=p[:], in_=ps_pair[hx][:, :],
                                             func=AF.Exp,
                                             bias=colneg_s[:, kt:kt + 1],
                                             scale=1.0)
                        nc.tensor.matmul(
                            aos[hx, q0][:, :],
                            Vm65[kt][:, 65 * h:65 * h + 65], p[:, :],
                            start=(kt == 0), stop=False)
            for hx in (0, 1):
                h = 2 * hp + hx
                po = 64 * hx
                hs = slice(65 * h, 65 * h + 65)
                for (q0, qw) in qch:
                    ao = aos[hx, q0]
                    nc.tensor.matmul(ao[:, :], vm65row[0:1, hs],
                                     wvec_s[0:1, q0:q0 + qw],
                                     start=False, stop=True)
                    rc = pden.tile([1, qw], F32R, tag="rc", bufs=4,
                                   name=f"rc{h}_{q0}", padded_shape=[1, 512])
                    with nc.allow_low_precision(
                            reason="f32r annotation; fp22 recip is ample"):
                        nc.vector.reciprocal(out=rc[:], in_=ao[64:65, :])
                    rbp = psB.tile([64, qw], F32, tag="rb", bufs=1,
                                   name=f"rbp{h}_{q0}", padded_shape=[64, 512])
                    nc.tensor.matmul(rbp[:, :], ones_s[0:1, 0:64], rc[:],
                                     start=True, stop=True)
                    rbs = pden.tile([64, qw], F32, tag="rbs", bufs=3,
                                    name=f"rbs{h}_{q0}",
                                    padded_shape=[64, 512])
                    nc.vector.tensor_copy(out=rbs[:], in_=rbp[:, :])
                    nc.vector.tensor_tensor(
                        out=attnorm[hp][po:po + 64, q0:q0 + qw],
                        in0=ao[0:64, :], in1=rbs[:], op=OP.mult)

        sAB.close()

        # ---------------- phase C: Wo both layouts ----------------
        sCD = ExitStack()
        pcd = sCD.enter_context(tc.tile_pool(name="pcd", bufs=1, side="right"))
        mhaT = [pcd.tile([128, L], F32R, name=f"mhaT{j}") for j in range(DT)]
        mhaN = [pcd.tile([128, D], F32, name=f"mhaN{q}") for q in range(8)]

        if has_g:
            nc.sync.dma_start(out=gb[:], in_=lng.to_broadcast([128, D]))
        if has_b:
            nc.sync.dma_start(out=bb[:], in_=lnb.to_broadcast([128, D]))
        if has_d2b:
            nc.sync.dma_start(out=d2bb[:], in_=d2b.to_broadcast([128, D]))
        nc.vector.memset(epst[:], EPS)
        nc.sync.dma_start(out=d1b_s[:], in_=d1b[:, :])

        sC = ExitStack()
        pc_w = sC.enter_context(tc.tile_pool(name="pc_w", bufs=1))
        psC = sC.enter_context(tc.tile_pool(name="psC", bufs=1, space="PSUM"))
        from concourse.masks import make_identity
        ident = pc_w.tile([128, 128], F32R, name="ident")
        nc.vector.memset(ident[:].bitcast(mybir.dt.uint32), 0)
        make_identity(nc, ident[:], nomemset=True)
        wos = [pc_w.tile([128, D], F32R, name=f"wos{t}") for t in range(DT)]
        for t in range(DT):
            dma_split(wos[t], wo[128 * t:128 * t + 128, :].bitcast(F32R), 3)
        for q0 in (0, 512):
            for j in range(DT):
                ps = psC.tile([128, 512], F32, tag="C", bufs=3,
                              name=f"psT{j}_{q0}", padded_shape=[128, 768])
                for t in range(DT):
                    nc.tensor.matmul(ps[:, :],
                                     wos[t][:, 128 * j:128 * j + 128],
                                     attnorm[t][:, q0:q0 + 512],
                                     start=(t == 0), stop=(t == DT - 1))
                nc.vector.tensor_copy(out=mhaT[j][:, q0:q0 + 512],
                                      in_=ps[:, :])
            for qi in range(q0 // 128, q0 // 128 + 4):
                # mha natural = blockwise PE transpose of mhaT
                for j in range(DT):
                    tp = psC.tile([128, 128], F32R, tag="tr", bufs=2,
                                  name=f"tp{qi}_{j}")
                    nc.tensor.transpose(
                        tp[:, :], mhaT[j][:, 128 * qi:128 * qi + 128],
                        ident[:])
                    dst = mhaN[qi][:, 128 * j:128 * j + 128]
                    if has_d2b:
                        nc.vector.tensor_tensor(out=dst, in0=tp[:, :],
                                                in1=d2bb[:, 128 * j:128 * j + 128],
                                                op=OP.add)
                    else:
                        nc.vector.tensor_copy(out=dst, in_=tp[:, :])
        sC.close()
        sBC.close()

        # ---------------- phase D: FFN + residual + LayerNorm ----------------
        sD = ExitStack()
        pd_w = sD.enter_context(tc.tile_pool(name="pd_w", bufs=1, side="right"))
        pdx = sD.enter_context(tc.tile_pool(name="pdx", bufs=1, side="right"))
        psmall = sD.enter_context(
            tc.tile_pool(name="psmall", bufs=8, side="right"))
        psD = sD.enter_context(tc.tile_pool(name="psD", bufs=1, space="PSUM"))

        reluT = [pdx.tile([128, L], F32R, name=f"reluT{j}") for j in range(DT)]
        d2s = [pd_w.tile([128, D], F32R, name=f"d2s{t}") for t in range(DT)]
        d1s = [pd_w.tile([128, D], F32R, name=f"d1s{t}") for t in range(DT)]
        for t in range(DT):
            dma_split(d2s[t], d2w[128 * t:128 * t + 128, :].bitcast(F32R), 3)
            dma_split(d1s[t], d1w[128 * t:128 * t + 128, :].bitcast(F32R), 3)

        inv_d = 1.0 / D

        def emit_ffn_ln(qi):
            ps = psD.tile([128, D], F32, tag="D", bufs=4, name=f"psff{qi}",
                          padded_shape=[128, 768])
            for (n0, nw) in ((0, 512), (512, 256)):
                for t in range(DT):
                    nc.tensor.matmul(ps[:, n0:n0 + nw],
                                     reluT[t][:, 128 * qi:128 * qi + 128],
                                     d2s[t][:, n0:n0 + nw],
                                     start=(t == 0), stop=(t == DT - 1))
            x = pdx.tile([128, D], F32, tag="x", bufs=2, name=f"x{qi}")
            xsum = psmall.tile([128, 1], F32, tag="s1", name=f"xsum{qi}")
            # fused: x = ffn + mha, xsum = row-sum(x), one DVE pass
            nc.vector.scalar_tensor_tensor(out=x[:], in0=ps[:, :], scalar=0.0,
                                           in1=mhaN[qi][:], op0=OP.bypass,
                                           op1=OP.add, accum_out=xsum[:])
            scr = pdx.tile([128, D], F32, tag="scr", bufs=2, name=f"scr{qi}")
            xsq = psmall.tile([128, 1], F32, tag="s2", name=f"xsq{qi}")
            nc.scalar.activation(out=scr[:], in_=x[:], func=AF.Square,
                                 accum_out=xsq[:])
            mu = psmall.tile([128, 1], F32, tag="s3", name=f"mu{qi}")
            nc.vector.tensor_scalar_mul(out=mu[:], in0=xsum[:], scalar1=inv_d)
            var = psmall.tile([128, 1], F32, tag="s4", name=f"var{qi}")
            # var = xsq/D - mu^2  ==  (xsq*1/D) - mu*mu
            mu2 = psmall.tile([128, 1], F32, tag="s5", name=f"mu2{qi}")
            nc.vector.tensor_tensor(out=mu2[:], in0=mu[:], in1=mu[:],
                                    op=OP.mult)
            nc.vector.scalar_tensor_tensor(out=var[:], in0=xsq[:],
                                           scalar=inv_d, in1=mu2[:],
                                           op0=OP.mult, op1=OP.subtract)
            std = psmall.tile([128, 1], F32, tag="s6", name=f"std{qi}")
            nc.scalar.activation(out=std[:], in_=var[:], func=AF.Sqrt,
                                 bias=epst[:], scale=1.0)
            rstd = psmall.tile([128, 1], F32, tag="s7", name=f"rstd{qi}")
            nc.vector.reciprocal(out=rstd[:], in_=std[:])
            nmb = psmall.tile([128, 1], F32, tag="s8", name=f"nmb{qi}")
            nc.vector.scalar_tensor_tensor(out=nmb[:], in0=mu[:], scalar=-1.0,
                                           in1=rstd[:], op0=OP.mult,
                                           op1=OP.mult)
            # xn = x*rstd + (-mu*rstd); optional *g (DVE) and +b (GpSimd)
            cur = scr
            nc.vector.tensor_scalar(out=cur[:], in0=x[:], scalar1=rstd[:],
                                    scalar2=nmb[:], op0=OP.mult, op1=OP.add)
            if has_g:
                nc.vector.tensor_tensor(out=x[:], in0=cur[:], in1=gb[:],
                                        op=OP.mult)
                cur = x
            if has_b:
                xo = pdx.tile([128, D], F32, tag="xo", bufs=2, name=f"xo{qi}")
                if qi >= 6:
                    nc.vector.tensor_tensor(out=xo[:], in0=cur[:], in1=bb[:],
                                            op=OP.add)
                else:
                    nc.gpsimd.tensor_tensor(out=xo[:], in0=cur[:], in1=bb[:],
                                            op=OP.add)
                cur = xo
            nc.sync.dma_start(out=out[128 * qi:128 * qi + 128, :],
                              in_=cur[:])

        for q0 in (0, 512):
            for j in range(DT):
                ps = psD.tile([128, 512], F32, tag="D", bufs=4,
                              name=f"psd1_{j}_{q0}", padded_shape=[128, 768])
                for t in range(DT):
                    nc.tensor.matmul(ps[:, :],
                                     d1s[t][:, 128 * j:128 * j + 128],
                                     mhaT[t][:, q0:q0 + 512],
                                     start=(t == 0), stop=(t == DT - 1))
                nc.scalar.activation(out=reluT[j][:, q0:q0 + 512],
                                     in_=ps[:, :], func=AF.Relu,
                                     bias=d1b_s[:, j:j + 1], scale=1.0)
            for qi in range(q0 // 128, q0 // 128 + 4):
                emit_ffn_ln(qi)
        sD.close()
        sCD.close()

    nc.compile()
    return nc


_PROGRAM_CACHE = {}


def _get_program(k_pad, q_pad, n_cores, has_g, has_b, has_d2b):
    key = (k_pad, q_pad, n_cores, has_g, has_b, has_d2b)
    if key not in _PROGRAM_CACHE:
        _PROGRAM_CACHE[key] = build_program(k_pad, q_pad, n_cores,
                                            has_g, has_b, has_d2b)
    return _PROGRAM_CACHE[key]


def make_in_map(b, k_pad, q_pad, queries, keys, values, mask_1, mask_2,
                Wq, Wk, Wv, Wo, d1_w, d1_b, d2_w, d2_b, ln_g, ln_b):
    kt_n = k_pad // 128
    f32 = np.float32
    vl1 = int(np.count_nonzero(mask_1[b]))
    vl2 = int(np.count_nonzero(mask_2[b]))
    row01 = (np.arange(L) < vl2).astype(f32)
    qmask = np.asarray(queries[b], f32) * row01[:, None]
    col01 = (np.arange(L) < vl1)
    cn = np.where(col01, 0.0, NEG).astype(f32)[:k_pad]
    vb = np.asarray(values[b], f32)
    vinmasked = vb[vl1:, :].sum(0, dtype=np.float64).astype(f32)
    vinall = vb.sum(0, dtype=np.float64).astype(f32)
    return {
        "qT": np.ascontiguousarray(qmask.T[:, :q_pad]),
        "kT": np.ascontiguousarray(np.asarray(keys[b], f32).T[:, :k_pad]),
        "vT": np.ascontiguousarray(vb.T[:, :k_pad]),
        "wq": np.ascontiguousarray(np.asarray(Wq, f32) * 0.125),
        "wk": np.ascontiguousarray(np.asarray(Wk, f32)),
        "wv": np.ascontiguousarray(np.asarray(Wv, f32)),
        "wo": np.ascontiguousarray(np.asarray(Wo, f32)),
        "d1w": np.ascontiguousarray(np.asarray(d1_w, f32)),
        "d2w": np.ascontiguousarray(np.asarray(d2_w, f32)),
        "vauxm": np.ascontiguousarray(vinmasked.reshape(DT, 128).T),
        "vauxa": np.ascontiguousarray(vinall.reshape(DT, 128).T),
        "colneg": np.ascontiguousarray(cn.reshape(kt_n, 128).T),
        "wvec": np.ascontiguousarray((1.0 - row01)[None, :q_pad]),
        "sigu": np.full((1, H), float(L - vl1), f32),
        "d1b": np.ascontiguousarray(np.asarray(d1_b, f32).reshape(DT, 128).T),
        "d2b": np.ascontiguousarray(np.asarray(d2_b, f32)[None, :]),
        "lng": np.ascontiguousarray(np.asarray(ln_g, f32)[None, :]),
        "lnb": np.ascontiguousarray(np.asarray(ln_b, f32)[None, :]),
    }


def kernel(queries, keys, values, mask_1, mask_2,
           Wq, Wk, Wv, Wo, d1_w, d1_b, d2_w, d2_b, ln_g, ln_b):
    from concourse.bass_utils import run_bass_kernel_spmd

    queries = np.asarray(queries)
    B = queries.shape[0]
    vl1 = np.count_nonzero(np.asarray(mask_1), axis=1)
    vl2 = np.count_nonzero(np.asarray(mask_2), axis=1)
    k_pad = _pad128(vl1.max())
    q_pad = _pad128(vl2.max())
    has_g = not np.all(np.asarray(ln_g) == 1.0)
    has_b = bool(np.any(np.asarray(ln_b)))
    has_d2b = bool(np.any(np.asarray(d2_b)))
    nc = _get_program(k_pad, q_pad, B, has_g, has_b, has_d2b)
    in_maps = [
        make_in_map(b, k_pad, q_pad, queries, keys, values, mask_1, mask_2,
                    Wq, Wk, Wv, Wo, d1_w, d1_b, d2_w, d2_b, ln_g, ln_b)
        for b in range(B)
    ]
    res = run_bass_kernel_spmd(nc, in_maps, list(range(B)))
    return np.stack([res.results[b]["out"] for b in range(B)], axis=0)
